# revision 1
# baseline (speedup 1.0000x reference)
# Trainium2 Bass kernel for DirectionalStockGNN (2-layer GATv2 + residual head).
#
# Sharding: edges are sorted by destination node on the host; each of the 8
# cores owns a contiguous range of N/8 destination nodes and all edges into
# them.  The segment softmax is then fully core-local (scores stay bounded,
# ~|e|<6, so no max-subtraction is needed).  Node features / weights are
# replicated; the only collective is an AllGather of the layer-1 hidden state
# between the two GAT layers.
#
# Per-core edge pipeline (feature-major, window = 124 consecutive dst nodes):
#   psum_m[f,t]  = xr[dst_t,f] + (ea@We)[t,f]      one matmul, stationary
#                                                   lhsT = [xr_win ; We]
#                + xl[src_t,f]                      PE transposes of rows
#                                                   fetched by dma_gather
#                                                   (int16 idx; table split at
#                                                   row HALF for range)
#   za           = 0.2*att  * psum_m   (ACT Copy, per-partition scale)
#   zr           = |att|    * psum_m   relu'd (ACT Relu)
#   score_t      = ones.za + (0.8*sign).zr          per-block matmuls + Exp
#   out[w,0:128]+= sum_t ee_t * onehot(dst_t) * xl[src_t]   (matmul)
#   out[w,128]  += sum_t ee_t * onehot(dst_t)               (ones matmul)
# followed by a small per-window epilogue (divide, bias, ELU, transpose).

import math
import os

import numpy as np

D = 128
DE = 4
WIN = 124
NEG = 0.2
HALF = 25000  # gather-table split row (int16 index range)


# ----------------------------------------------------------------------------
# host-side schedule + blob construction
# ----------------------------------------------------------------------------
def _wrap16(idx):
    """dma_gather index layout: [128, n/16] int16, wrap-16, replicated x8."""
    n = idx.shape[0]
    assert n % 16 == 0
    iw = np.zeros((16, n // 16), np.int16)
    iw[np.arange(n) % 16, np.arange(n) // 16] = idx
    return np.tile(iw, (8, 1))  # [128, n//16]


def build_host_data(x, edge_index, edge_attr, ncores):
    N = x.shape[0]
    src0 = np.asarray(edge_index[0], dtype=np.int64)
    dst0 = np.asarray(edge_index[1], dtype=np.int64)
    ea = np.asarray(edge_attr, dtype=np.float32)

    # self loops with mean edge_attr per dst (PyG fill_value='mean')
    sums = np.zeros((N, DE), np.float32)
    np.add.at(sums, dst0, ea)
    cnts = np.bincount(dst0, minlength=N).astype(np.float32)
    loop_attr = sums / np.maximum(cnts, 1.0)[:, None]

    src = np.concatenate([src0, np.arange(N, dtype=np.int64)])
    dst = np.concatenate([dst0, np.arange(N, dtype=np.int64)])
    eaa = np.concatenate([ea, loop_attr], axis=0)

    order = np.argsort(dst, kind="stable")
    src_s = src[order]
    dst_s = dst[order]
    ea_s = eaa[order]

    NPC = N // ncores
    NW = math.ceil(NPC / WIN)
    half = min(HALF, N)

    # per-core window edge ranges (common window grid)
    starts = np.minimum(np.arange(NW + 1) * WIN, NPC)
    bounds = np.empty((ncores, NW + 1), np.int64)
    for c in range(ncores):
        bounds[c] = np.searchsorted(dst_s, c * NPC + starts)

    # per (core, window): split edges into src<HALF and src>=HALF
    nlo = np.empty((ncores, NW), np.int64)
    nhi = np.empty((ncores, NW), np.int64)
    for c in range(ncores):
        for w in range(NW):
            lo, hi = bounds[c, w], bounds[c, w + 1]
            nlo[c, w] = int((src_s[lo:hi] < half).sum())
            nhi[c, w] = int(hi - lo - nlo[c, w])
    KWLO = np.ceil(nlo.max(axis=0) / 128.0).astype(np.int64)
    KWHI = np.ceil(nhi.max(axis=0) / 128.0).astype(np.int64)
    KWLO = np.maximum(KWLO, 1)  # >=1 so every window has at least one block

    blobI = []  # int16 gather indices (wrap-16 layout)
    blobA = []  # f32 dst_rel columns [128, kw]
    blobB = []  # f32 [5, ew]: dst_rel row + ea^T
    for c in range(ncores):
        irecs = []
        arecs = []
        brecs = []
        for w in range(NW):
            lo, hi = bounds[c, w], bounds[c, w + 1]
            kwlo, kwhi = int(KWLO[w]), int(KWHI[w])
            kw = kwlo + kwhi
            ew = kw * 128
            base = c * NPC + w * WIN
            sw = src_s[lo:hi]
            dw = (dst_s[lo:hi] - base).astype(np.float32)
            ew_ = ea_s[lo:hi]
            mlo = sw < half
            # low half then high half, each padded to its block count
            srcp = np.zeros(ew, np.int64)
            drel = np.full(ew, 127.0, np.float32)
            eap = np.zeros((ew, DE), np.float32)
            a = int(mlo.sum())
            srcp[:a] = sw[mlo]
            drel[:a] = dw[mlo]
            eap[:a] = ew_[mlo]
            b0 = kwlo * 128
            b = int((~mlo).sum())
            srcp[b0 : b0 + b] = sw[~mlo]
            drel[b0 : b0 + b] = dw[~mlo]
            eap[b0 : b0 + b] = ew_[~mlo]
            srcp[b0 + b :] = half  # high-half pads -> rel idx 0
            ilo = _wrap16(srcp[:b0].astype(np.int16))  # [128, 8*kwlo]
            if kwhi:
                ihi = _wrap16((srcp[b0:] - half).astype(np.int16))
                irecs.append(np.concatenate([ilo, ihi], axis=1))
            else:
                irecs.append(ilo)
            arecs.append(np.ascontiguousarray(drel.reshape(kw, 128).T))
            brecs.append(np.concatenate([drel[None, :], eap.T], axis=0))
        blobI.append(np.concatenate(irecs, axis=1))  # [128, 8*sumKW]
        blobA.append(np.concatenate(arecs, axis=1))  # [128, sumKW]
        blobB.append(np.concatenate(brecs, axis=1))  # [5, 128*sumKW]
    blobI = np.stack(blobI)
    blobA = np.stack(blobA)
    blobB = np.stack(blobB)

    KW = (KWLO + KWHI).astype(np.int64)
    koff = np.zeros(NW + 1, np.int64)  # cumulative blocks
    for w in range(NW):
        koff[w + 1] = koff[w] + int(KW[w])

    sched = dict(
        N=N, NPC=NPC, NW=NW,
        KWLO=[int(k) for k in KWLO], KWHI=[int(k) for k in KWHI],
        koff=[int(v) for v in koff], ncores=ncores, half=half,
    )
    return sched, blobI, blobA, blobB


def build_consts(ins):
    f32 = np.float32
    x = np.ascontiguousarray(np.asarray(ins["x"], f32))
    consts = {}
    consts["xT"] = np.ascontiguousarray(x.T)  # [128, N]
    for li in (1, 2):
        Wl = np.asarray(ins[f"W{li}l"], f32)
        Wr = np.asarray(ins[f"W{li}r"], f32)
        We = np.asarray(ins[f"W{li}e"], f32)
        a = np.asarray(ins[f"att{li}"], f32)
        consts[f"wl{li}"] = np.ascontiguousarray(Wl)
        consts[f"wr{li}"] = np.ascontiguousarray(Wr)
        consts[f"we{li}"] = np.ascontiguousarray(We)  # [4,128]
        consts[f"attabs{li}"] = np.ascontiguousarray(np.abs(a)[:, None])
        consts[f"att02_{li}"] = np.ascontiguousarray(NEG * a[:, None])
        consts[f"sgn{li}"] = np.ascontiguousarray(
            ((1.0 - NEG) * np.sign(a))[:, None]
        )
        b = np.asarray(ins[f"b{li}"], f32)
        consts[f"bb{li}"] = np.ascontiguousarray(np.tile(b[None, :], (D, 1)))
    consts["wfc"] = np.ascontiguousarray(np.asarray(ins["Wfc"], f32).reshape(D, 1))
    consts["iota_bc"] = np.ascontiguousarray(
        np.tile(np.arange(WIN, dtype=f32)[None, :], (D, 1))
    )
    consts["iotacol"] = np.arange(WIN, dtype=f32)[:, None].copy()
    consts["ones1"] = np.ones((1, WIN), f32)
    consts["onec"] = np.ones((D, 1), f32)
    consts["zcol"] = np.zeros((D, 1), f32)
    consts["ident"] = np.eye(D, dtype=f32)
    return consts


# ----------------------------------------------------------------------------
# bass program
# ----------------------------------------------------------------------------
def build_program(sched, bfc_val):
    import concourse.bacc as bacc
    import concourse.bass as bass
    import concourse.mybir as mybir
    import concourse.tile as tile

    f32 = mybir.dt.float32
    i16 = mybir.dt.int16
    Alu = mybir.AluOpType
    Act = mybir.ActivationFunctionType

    ncores = sched["ncores"]
    N, NPC, NW = sched["N"], sched["NPC"], sched["NW"]
    KWLO, KWHI = sched["KWLO"], sched["KWHI"]
    koff = sched["koff"]
    half = sched["half"]
    KW = [KWLO[w] + KWHI[w] for w in range(NW)]
    KWMAX = max(KW)
    EWMAX = KWMAX * 128
    HT = NW * WIN

    nc = bacc.Bacc(
        "TRN2", target_bir_lowering=False, debug=False,
        enable_asserts=False, num_devices=ncores,
    )

    # ---- I/O ----
    t_xT = nc.dram_tensor("xT", [D, N], f32, kind="ExternalInput")
    t_xT_own = nc.dram_tensor("xT_own", [D, NPC], f32, kind="ExternalInput")
    KTOT = koff[NW]
    t_blobI = nc.dram_tensor("blobI", [128, 8 * KTOT], i16, kind="ExternalInput")
    t_blobA = nc.dram_tensor("blobA", [128, KTOT], f32, kind="ExternalInput")
    t_blobB = nc.dram_tensor("blobB", [5, 128 * KTOT], f32, kind="ExternalInput")
    cshapes = dict(
        wl1=[D, D], wr1=[D, D], wl2=[D, D], wr2=[D, D],
        we1=[DE, D], we2=[DE, D],
        attabs1=[D, 1], att02_1=[D, 1], sgn1=[D, 1],
        attabs2=[D, 1], att02_2=[D, 1], sgn2=[D, 1],
        bb1=[D, D], bb2=[D, D], wfc=[D, 1],
        iota_bc=[D, WIN], iotacol=[WIN, 1], ones1=[1, WIN],
        onec=[D, 1], zcol=[D, 1], ident=[D, D],
    )
    t_c = {k: nc.dram_tensor(k, sh, f32, kind="ExternalInput") for k, sh in cshapes.items()}
    t_y = nc.dram_tensor("y", [NPC, 1], f32, kind="ExternalOutput")
    t_dbg = None
    if os.environ.get("GNN_DBG"):
        t_dbg = nc.dram_tensor("dbg", [ncores, D, NPC], f32, kind="ExternalOutput")

    # ---- DRAM internals ----
    t_tab1 = nc.dram_tensor("tab1", [N, D], f32, kind="Internal")
    t_tab2 = nc.dram_tensor("tab2", [N, D], f32, kind="Internal")
    t_h1T_own = nc.dram_tensor("h1T_own", [D, NPC], f32, kind="Internal")
    t_h1T_all = nc.dram_tensor(
        "h1T_all", [ncores, D, NPC], f32, kind="Internal",
        addr_space=("Shared" if ncores > 1 else "Local"),
    )

    with tile.TileContext(nc) as tc:
        with (
            tc.tile_pool(name="cpool", bufs=1) as cpool,
            tc.tile_pool(name="sp", bufs=3) as sp,
            tc.tile_pool(name="sp2", bufs=2) as sp2,
            tc.tile_pool(name="pm", bufs=2, space="PSUM") as pm_pool,
            tc.tile_pool(name="pbc", bufs=1, space="PSUM") as pbc_pool,
            tc.tile_pool(name="pe", bufs=1, space="PSUM") as pe_pool,
            tc.tile_pool(name="pwin", bufs=2, space="PSUM") as pwin_pool,
            tc.tile_pool(name="paux", bufs=1, space="PSUM") as paux_pool,
            tc.tile_pool(name="pden", bufs=1, space="PSUM") as pden_pool,
        ):
            # ---- load consts ----
            C = {}
            for k, sh in cshapes.items():
                C[k] = cpool.tile(sh, f32, tag=f"c_{k}", name=f"c_{k}")
                nc.sync.dma_start(out=C[k][:], in_=t_c[k][:])

            lhsT_sb = cpool.tile([D, NW, D], f32, tag="lhsT_sb", name="lhsT_sb")
            hT_res = cpool.tile([D, HT], f32, tag="hT_res", name="hT_res")
            y_sb = cpool.tile([1, HT], f32, tag="y_sb", name="y_sb")

            def dense_table(layer, t_tab):
                wl = C[f"wl{layer}"]
                if layer == 1:
                    srcs = [(None, 0, N)]
                else:
                    srcs = [(c8, c8 * NPC, NPC) for c8 in range(ncores)]
                for c8, gbase, nrows in srcs:
                    for r0 in range(0, nrows, 128):
                        rn = min(128, nrows - r0)
                        xt_t = sp.tile([D, 128], f32, tag="xt_t", name="xt_t")
                        if layer == 1:
                            nc.sync.dma_start(out=xt_t[:, :rn], in_=t_xT[:, r0 : r0 + rn])
                        else:
                            nc.sync.dma_start(
                                out=xt_t[:, :rn], in_=t_h1T_all[c8, :, r0 : r0 + rn]
                            )
                        ps = paux_pool.tile([D, 129], f32, tag="paux", name="ps")
                        nc.tensor.matmul(
                            out=ps[:rn, :128], lhsT=xt_t[:, :rn], rhs=wl[:, :],
                            start=True, stop=True,
                        )
                        stg = sp.tile([D, D], f32, tag="stg", name="stg")
                        nc.scalar.copy(out=stg[:rn, :], in_=ps[:rn, :128])
                        nc.sync.dma_start(
                            out=t_tab[gbase + r0 : gbase + r0 + rn, :], in_=stg[:rn, :]
                        )

            def dense_xr(layer):
                wr = C[f"wr{layer}"]
                nc.vector.memset(lhsT_sb[:, :, :], 0.0)
                for w in range(NW):
                    wn = min(WIN, NPC - w * WIN)
                    if layer == 1:
                        xt_t = sp.tile([D, WIN], f32, tag="xt_w", name="xt_w")
                        nc.sync.dma_start(
                            out=xt_t[:, :wn], in_=t_xT_own[:, w * WIN : w * WIN + wn]
                        )
                        lhs = xt_t[:, :wn]
                    else:
                        lhs = hT_res[:, w * WIN : w * WIN + wn]
                    ps = paux_pool.tile([D, 129], f32, tag="paux", name="psx")
                    nc.tensor.matmul(
                        out=ps[:wn, :128], lhsT=lhs, rhs=wr[:, :],
                        start=True, stop=True,
                    )
                    nc.scalar.copy(out=lhsT_sb[:wn, w, 0:128], in_=ps[:wn, :128])
                    nc.sync.dma_start(
                        out=lhsT_sb[124:128, w, 0:128], in_=t_c[f"we{layer}"][:, :]
                    )

            def edge_pass(layer, t_tab):
                eplvl = int(os.environ.get("GNN_EP_LVL", "4"))
                attabs = C[f"attabs{layer}"]
                att02 = C[f"att02_{layer}"]
                sgn = C[f"sgn{layer}"]
                if eplvl < 4 and layer == 1:
                    nc.vector.memset(hT_res[:, :], 0.0)
                for w in range(NW):
                    kwlo, kwhi = KWLO[w], KWHI[w]
                    kw = kwlo + kwhi
                    ew = kw * 128
                    wn = min(WIN, NPC - w * WIN)
                    ko = koff[w]
                    it = sp2.tile([D, 8 * KWMAX], i16, tag="it", name="it")
                    nc.sync.dma_start(
                        out=it[:, : 8 * kw],
                        in_=t_blobI[:, 8 * ko : 8 * ko + 8 * kw],
                    )
                    at = sp2.tile([D, KWMAX], f32, tag="at", name="at")
                    nc.sync.dma_start(
                        out=at[:, :kw], in_=t_blobA[:, ko : ko + kw]
                    )
                    dstrow = sp2.tile([1, EWMAX], f32, tag="dstrow", name="dstrow")
                    nc.sync.dma_start(
                        out=dstrow[:, :ew],
                        in_=t_blobB[0:1, 128 * ko : 128 * ko + ew],
                    )
                    rhs_t = sp2.tile([D, EWMAX], f32, tag="rhs_t", name="rhs_t")
                    nc.sync.dma_start(
                        out=rhs_t[124:128, :ew],
                        in_=t_blobB[1:5, 128 * ko : 128 * ko + ew],
                    )
                    xg = sp2.tile([D, KWMAX, D], f32, tag="xg", name="xg")
                    CH = 8  # blocks per dma_gather call (1024 idxs max safe)

                    def do_gathers(base_blk, nblk, tab_ap, icol0):
                        for g0 in range(0, nblk, CH):
                            gn = min(CH, nblk - g0)
                            nc.gpsimd.dma_gather(
                                out_ap=xg[:, base_blk + g0 : base_blk + g0 + gn, :],
                                in_ap=tab_ap,
                                idxs_ap=it[:, icol0 + 8 * g0 : icol0 + 8 * (g0 + gn)],
                                num_idxs=gn * 128,
                                num_idxs_reg=gn * 128,
                                elem_size=D,
                            )

                    do_gathers(0, kwlo, t_tab[0:half, :], 0)
                    if kwhi:
                        do_gathers(kwlo, kwhi, t_tab[half:N, :], 8 * kwlo)
                    if eplvl < 2:
                        continue
                    pwin = pwin_pool.tile([D, 129], f32, tag="pwin", name="pwin")
                    pden = pden_pool.tile([D, 1], f32, tag="pden", name="pden")
                    nblk_done = 0
                    for t0 in range(0, kw, 4):
                        nb = min(4, kw - t0)
                        T = nb * 128
                        c0 = t0 * 128
                        # dst_rel broadcast to 124 partitions (K=1 matmul)
                        pbc = pbc_pool.tile([WIN, 512], f32, tag="pbc", name="pbc")
                        nc.tensor.matmul(
                            out=pbc[:, :T], lhsT=C["ones1"][:, :],
                            rhs=dstrow[:, c0 : c0 + T], start=True, stop=True,
                        )
                        nc.vector.tensor_scalar(
                            out=rhs_t[0:124, c0 : c0 + T], in0=pbc[:, :T],
                            scalar1=C["iotacol"][:, :], scalar2=None,
                            op0=Alu.is_equal,
                        )
                        # m = xr[dst] + ea@We (+ xl[src] via transposes)
                        pm = pm_pool.tile([D, 512], f32, tag="pm", name="pm")
                        nc.tensor.matmul(
                            out=pm[:, :T], lhsT=lhsT_sb[:, w, :],
                            rhs=rhs_t[:, c0 : c0 + T], start=True, stop=False,
                        )
                        for cb in range(nb):
                            nc.tensor.matmul(
                                out=pm[:, cb * 128 : (cb + 1) * 128],
                                lhsT=xg[:, t0 + cb, :],
                                rhs=C["ident"][:, :],
                                is_transpose=True,
                                start=False, stop=(cb == nb - 1),
                            )
                        # za = 0.2*att*m ; zr = relu(|att|*m)
                        za = sp.tile([D, 512], f32, tag="za", name="za")
                        nc.scalar.activation(
                            out=za[:, :T], in_=pm[:, :T], func=Act.Copy,
                            scale=att02[:, :],
                        )
                        zr = sp.tile([D, 512], f32, tag="zr", name="zr")
                        nc.scalar.activation(
                            out=zr[:, :T], in_=pm[:, :T], func=Act.Relu,
                            scale=attabs[:, :], bias=C["zcol"][:, :],
                        )
                        if eplvl < 3:
                            continue
                        # scores + exp
                        pev = pe_pool.tile([D, 4], f32, tag="pe", name="pev")
                        for cb in range(nb):
                            nc.tensor.matmul(
                                out=pev[:, cb : cb + 1],
                                lhsT=za[:, cb * 128 : (cb + 1) * 128],
                                rhs=C["onec"][:, :],
                                start=True, stop=False,
                            )
                            nc.tensor.matmul(
                                out=pev[:, cb : cb + 1],
                                lhsT=zr[:, cb * 128 : (cb + 1) * 128],
                                rhs=sgn[:, :],
                                start=False, stop=True,
                            )
                        ee = sp.tile([D, 4], f32, tag="ee", name="ee")
                        nc.scalar.activation(
                            out=ee[:, :nb], in_=pev[:, :nb],
                            func=Act.Exp, bias=C["zcol"][:, :],
                        )
                        # S_ee and aggregation (+ denominator via ones rhs)
                        see = sp.tile([D, 4 * WIN], f32, tag="see", name="see")
                        for cb in range(nb):
                            nc.vector.tensor_scalar(
                                out=see[:, cb * WIN : (cb + 1) * WIN],
                                in0=C["iota_bc"][:, :],
                                scalar1=at[:, t0 + cb : t0 + cb + 1],
                                scalar2=ee[:, cb : cb + 1],
                                op0=Alu.is_equal, op1=Alu.mult,
                            )
                        if eplvl < 4:
                            continue
                        for cb in range(nb):
                            glob_b = nblk_done + cb
                            nc.tensor.matmul(
                                out=pwin[0:WIN, 0:128],
                                lhsT=see[:, cb * WIN : (cb + 1) * WIN],
                                rhs=xg[:, t0 + cb, :],
                                start=(glob_b == 0), stop=(glob_b == kw - 1),
                            )
                            nc.tensor.matmul(
                                out=pden[0:WIN, 0:1],
                                lhsT=see[:, cb * WIN : (cb + 1) * WIN],
                                rhs=C["onec"][:, :],
                                start=(glob_b == 0), stop=(glob_b == kw - 1),
                            )
                        nblk_done += nb
                    # ---- window epilogue ----
                    if eplvl < 4:
                        continue
                    den = sp.tile([WIN, 1], f32, tag="den", name="den")
                    nc.vector.tensor_scalar(
                        out=den[:, :], in0=pden[0:WIN, 0:1],
                        scalar1=1e-30, scalar2=None, op0=Alu.max,
                    )
                    rec = sp.tile([WIN, 1], f32, tag="rec", name="rec")
                    nc.vector.reciprocal(out=rec[:, :], in_=den[:, :])
                    hw_ = sp.tile([WIN, D], f32, tag="hw", name="hw_")
                    nc.vector.tensor_scalar(
                        out=hw_[:, :], in0=pwin[0:WIN, 0:128],
                        scalar1=rec[:, :], scalar2=None, op0=Alu.mult,
                    )
                    nc.vector.tensor_tensor(
                        out=hw_[:, :], in0=hw_[:, :], in1=C[f"bb{layer}"][0:WIN, :],
                        op=Alu.add,
                    )
                    # ELU: h - min(h,0) + exp(min(h,0)) - 1
                    tmin = sp.tile([WIN, D], f32, tag="tmin", name="tmin")
                    nc.vector.tensor_scalar(
                        out=tmin[:, :], in0=hw_[:, :], scalar1=0.0, scalar2=None,
                        op0=Alu.min,
                    )
                    uexp = sp.tile([WIN, D], f32, tag="uexp", name="uexp")
                    nc.scalar.activation(
                        out=uexp[:, :], in_=tmin[:, :], func=Act.Exp,
                        bias=C["zcol"][0:WIN, :],
                    )
                    nc.vector.tensor_tensor(
                        out=hw_[:, :], in0=hw_[:, :], in1=tmin[:, :], op=Alu.subtract
                    )
                    nc.vector.tensor_scalar(
                        out=uexp[:, :], in0=uexp[:, :], scalar1=-1.0, scalar2=None,
                        op0=Alu.add,
                    )
                    nc.vector.tensor_tensor(
                        out=hw_[:, :], in0=hw_[:, :], in1=uexp[:, :], op=Alu.add
                    )
                    # transpose h window -> [128f, wn]
                    pt = paux_pool.tile([D, 129], f32, tag="paux", name="pt")
                    nc.tensor.matmul(
                        out=pt[:, 0:WIN], lhsT=hw_[:, :], rhs=C["ident"][0:WIN, 0:WIN],
                        is_transpose=True, start=True, stop=True,
                    )
                    if layer == 1:
                        nc.scalar.copy(
                            out=hT_res[:, w * WIN : w * WIN + WIN], in_=pt[:, 0:WIN]
                        )
                    else:
                        h2t = sp.tile([D, WIN], f32, tag="h2t", name="h2t")
                        nc.scalar.copy(out=h2t[:, :], in_=pt[:, 0:WIN])
                        xt_f = sp.tile([D, WIN], f32, tag="xt_fin", name="xt_f")
                        nc.sync.dma_start(
                            out=xt_f[:, :wn], in_=t_xT_own[:, w * WIN : w * WIN + wn]
                        )
                        nc.vector.tensor_tensor(
                            out=h2t[:, :wn], in0=h2t[:, :wn], in1=xt_f[:, :wn],
                            op=Alu.add,
                        )
                        py = paux_pool.tile([D, 129], f32, tag="paux", name="py")
                        nc.tensor.matmul(
                            out=py[0:1, :wn], lhsT=C["wfc"][:, :], rhs=h2t[:, :wn],
                            start=True, stop=True,
                        )
                        nc.scalar.activation(
                            out=y_sb[:, w * WIN : w * WIN + wn], in_=py[0:1, :wn],
                            func=Act.Copy, bias=float(bfc_val),
                        )

            # ---------------- phases (GNN_MAXPHASE truncates for bisect) ----
            maxphase = int(os.environ.get("GNN_MAXPHASE", "6"))
            timeloop = int(os.environ.get("GNN_TIMELOOP", "0"))

            def body():
                if maxphase < 6:
                    nc.vector.memset(y_sb[:, :], 0.0)
                dense_table(1, t_tab1)
                if maxphase >= 1:
                    dense_xr(1)
                if maxphase >= 2:
                    edge_pass(1, t_tab1)
                    nc.sync.dma_start(out=t_h1T_own[:, :], in_=hT_res[:, 0:NPC])
                if maxphase >= 3:
                    if ncores > 1 and not timeloop:
                        nc.gpsimd.collective_compute(
                            "AllGather",
                            mybir.AluOpType.bypass,
                            replica_groups=[list(range(ncores))],
                            ins=[t_h1T_own[:, :]],
                            outs=[t_h1T_all[:, :, :]],
                        )
                    else:
                        for c8 in range(min(ncores, 1 if not timeloop else ncores)):
                            nc.sync.dma_start(
                                out=t_h1T_all[c8, :, :], in_=t_h1T_own[:, :]
                            )
                if maxphase >= 4:
                    dense_table(2, t_tab2)
                if maxphase >= 5:
                    dense_xr(2)
                if maxphase >= 6:
                    edge_pass(2, t_tab2)

            if timeloop:
                with tc.For_i(0, timeloop, 1):
                    body()
            else:
                body()
            if t_dbg is not None and maxphase >= 3:
                for c8 in range(ncores):
                    for r0 in range(0, NPC, 512):
                        rn = min(512, NPC - r0)
                        dbg_t = sp.tile([D, 512], f32, tag="dbg_t", name="dbg_t")
                        nc.sync.dma_start(
                            out=dbg_t[:, :rn], in_=t_h1T_all[c8, :, r0 : r0 + rn]
                        )
                        nc.sync.dma_start(
                            out=t_dbg[c8, :, r0 : r0 + rn], in_=dbg_t[:, :rn]
                        )
            nc.sync.dma_start(out=t_y[:, 0], in_=y_sb[0:1, 0:NPC])

    nc.compile()
    return nc


# ----------------------------------------------------------------------------
# entry points
# ----------------------------------------------------------------------------
def prepare(inputs, ncores=8):
    x = np.asarray(inputs["x"], np.float32)
    sched, blobI, blobA, blobB = build_host_data(
        x, inputs["edge_index"], inputs["edge_attr"], ncores
    )
    consts = build_consts(inputs)
    bfc_val = float(np.asarray(inputs["bfc"]).reshape(-1)[0])
    nc = build_program(sched, bfc_val)
    NPC = sched["NPC"]
    in_maps = []
    for c in range(ncores):
        m = dict(consts)
        m["xT_own"] = np.ascontiguousarray(consts["xT"][:, c * NPC : (c + 1) * NPC])
        m["blobI"] = np.ascontiguousarray(blobI[c])
        m["blobA"] = np.ascontiguousarray(blobA[c])
        m["blobB"] = np.ascontiguousarray(blobB[c])
        in_maps.append(m)
    return nc, in_maps, sched


def kernel(**inputs) -> np.ndarray:
    ncores = 8
    nc, in_maps, sched = prepare(inputs, ncores)
    from concourse.bass_utils import run_bass_kernel_spmd

    res = run_bass_kernel_spmd(nc, in_maps, core_ids=list(range(ncores)))
    y = np.concatenate([res.results[c]["y"] for c in range(ncores)], axis=0)
    return y.astype(np.float32)



# revision 12
# speedup vs baseline: 2.1185x; 2.1185x over previous
# Trainium2 Bass kernel for DirectionalStockGNN (2-layer GATv2 + residual head).
#
# Sharding: edges are sorted by destination node on the host; each of the 8
# cores owns a contiguous range of N/8 destination nodes and all edges into
# them.  The segment softmax is then fully core-local (scores stay bounded,
# ~|e|<15, so no max-subtraction is needed).  Node features / weights are
# replicated; the only collective is an AllGather of the layer-1 hidden state
# between the two GAT layers.
#
# All matmul operands are bf16 (PSUM accumulation fp32).  Per-core edge
# pipeline (feature-major, window = 124 consecutive dst nodes, block = 128
# edges, group = 4 blocks):
#   pm[f,t]   = xr[dst_t,f] + (ea@We)[t,f]    one matmul per group, stationary
#                                              lhsT=[xr_win;We], rhs = blobR
#                                              (host-built [onehot_dt ; ea^T])
#             + xl[src_t,f]                    PE transposes of dma_gather rows
#   lr        = Lrelu(|att| * pm)              one ACT pass per group
#   e_t       = sgn(att)^T lr_blk              one matmul per block
#   ee        = Exp(e)                         ACT per group
#   xgs       = [ee_t * xl[src_t] | ee_t]      DVE scale per block  [128,129]
#   pwin[d,:]+= onehot_td_blk^T @ xgs          one matmul per block; col 128
#                                              accumulates the softmax denom
# followed by a small per-window epilogue (reciprocal, bias, ELU, transpose).

import math
import os

import numpy as np
import ml_dtypes

BF16 = ml_dtypes.bfloat16

D = 128
DE = 4
WIN = 124
NEG = 0.2
HALF = 25000  # gather-table split row (int16 index range)


# ----------------------------------------------------------------------------
# host-side schedule + blob construction
# ----------------------------------------------------------------------------
def _wrap16(idx):
    """dma_gather index layout: [128, n/16] int16, wrap-16, replicated x8."""
    n = idx.shape[0]
    assert n % 16 == 0
    iw = np.zeros((16, n // 16), np.int16)
    iw[np.arange(n) % 16, np.arange(n) // 16] = idx
    return np.tile(iw, (8, 1))  # [128, n//16]


def build_host_data(x, edge_index, edge_attr, ncores):
    N = x.shape[0]
    src0 = np.asarray(edge_index[0], dtype=np.int64)
    dst0 = np.asarray(edge_index[1], dtype=np.int64)
    ea = np.asarray(edge_attr, dtype=np.float32)

    # self loops with mean edge_attr per dst (PyG fill_value='mean')
    sums = np.zeros((N, DE), np.float32)
    np.add.at(sums, dst0, ea)
    cnts = np.bincount(dst0, minlength=N).astype(np.float32)
    loop_attr = sums / np.maximum(cnts, 1.0)[:, None]

    src = np.concatenate([src0, np.arange(N, dtype=np.int64)])
    dst = np.concatenate([dst0, np.arange(N, dtype=np.int64)])
    eaa = np.concatenate([ea, loop_attr], axis=0)

    order = np.argsort(dst, kind="stable")
    src_s = src[order]
    dst_s = dst[order]
    ea_s = eaa[order]

    NPC = N // ncores
    NW = math.ceil(NPC / WIN)
    half = min(HALF, N)

    # per-core window edge ranges (common window grid)
    starts = np.minimum(np.arange(NW + 1) * WIN, NPC)
    bounds = np.empty((ncores, NW + 1), np.int64)
    for c in range(ncores):
        bounds[c] = np.searchsorted(dst_s, c * NPC + starts)

    # per (core, window): split edges into src<HALF and src>=HALF
    nlo = np.empty((ncores, NW), np.int64)
    nhi = np.empty((ncores, NW), np.int64)
    for c in range(ncores):
        for w in range(NW):
            lo, hi = bounds[c, w], bounds[c, w + 1]
            nlo[c, w] = int((src_s[lo:hi] < half).sum())
            nhi[c, w] = int(hi - lo - nlo[c, w])
    KWLO = np.ceil(nlo.max(axis=0) / 128.0).astype(np.int64)
    KWHI = np.ceil(nhi.max(axis=0) / 128.0).astype(np.int64)
    KWLO = np.maximum(KWLO, 1)  # >=1 so every window has at least one block

    KW = (KWLO + KWHI).astype(np.int64)
    koff = np.zeros(NW + 1, np.int64)  # cumulative blocks
    for w in range(NW):
        koff[w + 1] = koff[w] + int(KW[w])
    KTOT = int(koff[NW])

    blobI = np.zeros((ncores, 128, 8 * KTOT), np.int16)
    blobR = np.zeros((ncores, 128, 128 * KTOT), BF16)  # [onehot_dt ; ea^T]
    blobO = np.zeros((ncores, 128, 124 * KTOT), BF16)  # onehot_td
    drng = np.arange(WIN)
    for c in range(ncores):
        for w in range(NW):
            lo, hi = bounds[c, w], bounds[c, w + 1]
            kwlo, kwhi = int(KWLO[w]), int(KWHI[w])
            kw = kwlo + kwhi
            ew = kw * 128
            ko = int(koff[w])
            base = c * NPC + w * WIN
            sw = src_s[lo:hi]
            dw = (dst_s[lo:hi] - base).astype(np.int64)
            ew_ = ea_s[lo:hi]
            mlo = sw < half
            # low half then high half, each padded to its block count
            srcp = np.zeros(ew, np.int64)
            drel = np.full(ew, 127, np.int64)  # pad marker (no onehot row)
            eap = np.zeros((ew, DE), np.float32)
            a = int(mlo.sum())
            srcp[:a] = sw[mlo]
            drel[:a] = dw[mlo]
            eap[:a] = ew_[mlo]
            b0 = kwlo * 128
            b = int((~mlo).sum())
            srcp[b0 : b0 + b] = sw[~mlo]
            drel[b0 : b0 + b] = dw[~mlo]
            eap[b0 : b0 + b] = ew_[~mlo]
            srcp[b0 + b :] = half  # high-half pads -> rel idx 0
            ilo = _wrap16(srcp[:b0].astype(np.int16))  # [128, 8*kwlo]
            if kwhi:
                ihi = _wrap16((srcp[b0:] - half).astype(np.int16))
                blobI[c, :, 8 * ko : 8 * (ko + kw)] = np.concatenate(
                    [ilo, ihi], axis=1
                )
            else:
                blobI[c, :, 8 * ko : 8 * (ko + kw)] = ilo
            # onehot (both orientations) + ea rows
            oh = (drel[None, :] == drng[:, None]).astype(np.float32)  # [124,ew]
            rblk = np.zeros((128, ew), np.float32)
            rblk[0:WIN, :] = oh
            rblk[WIN : WIN + DE, :] = eap.T
            blobR[c, :, 128 * ko : 128 * ko + ew] = rblk.astype(BF16)
            # [t, d] orientation, per block contiguous: [128, kw*124]
            ot = np.ascontiguousarray(
                oh.T.reshape(kw, 128, WIN).transpose(1, 0, 2).reshape(128, kw * WIN)
            )
            blobO[c, :, 124 * ko : 124 * (ko + kw)] = ot.astype(BF16)

    sched = dict(
        N=N, NPC=NPC, NW=NW,
        KWLO=[int(k) for k in KWLO], KWHI=[int(k) for k in KWHI],
        koff=[int(v) for v in koff], ncores=ncores, half=half,
    )
    return sched, blobI, blobR, blobO


def build_consts(ins):
    f32 = np.float32
    x = np.ascontiguousarray(np.asarray(ins["x"], f32))
    consts = {}
    consts["xT"] = np.ascontiguousarray(x.T.astype(BF16))  # [128, N] bf16
    for li in (1, 2):
        Wl = np.asarray(ins[f"W{li}l"], f32)
        Wr = np.asarray(ins[f"W{li}r"], f32)
        We = np.asarray(ins[f"W{li}e"], f32)
        a = np.asarray(ins[f"att{li}"], f32)
        consts[f"wl{li}"] = np.ascontiguousarray(Wl.astype(BF16))
        consts[f"wr{li}"] = np.ascontiguousarray(Wr.astype(BF16))
        consts[f"we{li}"] = np.ascontiguousarray(We.astype(BF16))  # [4,128]
        consts[f"attabs{li}"] = np.ascontiguousarray(np.abs(a)[:, None])  # f32
        consts[f"sgnc{li}"] = np.ascontiguousarray(
            np.sign(a)[:, None].astype(BF16)
        )
        consts[f"att02_{li}"] = np.ascontiguousarray(NEG * a[:, None])  # f32
        consts[f"sgn08_{li}"] = np.ascontiguousarray(
            ((1.0 - NEG) * np.sign(a))[:, None].astype(BF16)
        )
        b = np.asarray(ins[f"b{li}"], f32)
        consts[f"bb{li}"] = np.ascontiguousarray(np.tile(b[None, :], (WIN, 1)))
    consts["wfc"] = np.ascontiguousarray(
        np.asarray(ins["Wfc"], f32).reshape(D, 1).astype(BF16)
    )
    consts["onecb"] = np.ones((D, 1), BF16)
    consts["identb"] = np.eye(D, dtype=BF16)
    consts["identf"] = np.eye(D, dtype=f32)
    return consts


# ----------------------------------------------------------------------------
# bass program
# ----------------------------------------------------------------------------
def build_program(sched, bfc_adj):
    import concourse.bacc as bacc
    import concourse.bass as bass
    import concourse.mybir as mybir
    import concourse.tile as tile

    f32 = mybir.dt.float32
    bf16 = mybir.dt.bfloat16
    i16 = mybir.dt.int16
    Alu = mybir.AluOpType
    Act = mybir.ActivationFunctionType

    ncores = sched["ncores"]
    N, NPC, NW = sched["N"], sched["NPC"], sched["NW"]
    KWLO, KWHI = sched["KWLO"], sched["KWHI"]
    koff = sched["koff"]
    half = sched["half"]
    KW = [KWLO[w] + KWHI[w] for w in range(NW)]
    KWMAX = max(KW)
    EWMAX = KWMAX * 128
    HT = NW * WIN
    KTOT = koff[NW]

    nc = bacc.Bacc(
        "TRN2", target_bir_lowering=False, debug=False,
        enable_asserts=False, num_devices=ncores,
    )

    # ---- I/O ----
    t_xT = nc.dram_tensor("xT", [D, N], bf16, kind="ExternalInput")
    t_xT_own = nc.dram_tensor("xT_own", [D, NPC], bf16, kind="ExternalInput")
    t_blobI = nc.dram_tensor("blobI", [128, 8 * KTOT], i16, kind="ExternalInput")
    t_blobR = nc.dram_tensor("blobR", [128, 128 * KTOT], bf16, kind="ExternalInput")
    t_blobO = nc.dram_tensor("blobO", [128, 124 * KTOT], bf16, kind="ExternalInput")
    cshapes = dict(
        wl1=([D, D], bf16), wr1=([D, D], bf16),
        wl2=([D, D], bf16), wr2=([D, D], bf16),
        we1=([DE, D], bf16), we2=([DE, D], bf16),
        attabs1=([D, 1], f32), sgnc1=([D, 1], bf16),
        attabs2=([D, 1], f32), sgnc2=([D, 1], bf16),
        att02_1=([D, 1], f32), att02_2=([D, 1], f32),
        sgn08_1=([D, 1], bf16), sgn08_2=([D, 1], bf16),
        onecb=([D, 1], bf16),
        bb1=([WIN, D], f32), bb2=([WIN, D], f32),
        wfc=([D, 1], bf16), identb=([D, D], bf16), identf=([D, D], f32),
    )
    t_c = {k: nc.dram_tensor(k, sh, dt, kind="ExternalInput")
           for k, (sh, dt) in cshapes.items()}
    t_y = nc.dram_tensor("y", [NPC, 1], f32, kind="ExternalOutput")

    # ---- DRAM internals ----
    t_tab1 = nc.dram_tensor("tab1", [N, D], bf16, kind="Internal")
    t_tab2 = nc.dram_tensor("tab2", [N, D], bf16, kind="Internal")
    t_h1T_own = nc.dram_tensor("h1T_own", [D, NPC], bf16, kind="Internal")
    t_h1T_all = nc.dram_tensor(
        "h1T_all", [ncores, D, NPC], bf16, kind="Internal",
        addr_space=("Shared" if ncores > 1 else "Local"),
    )

    with tile.TileContext(nc) as tc:
        with (
            tc.tile_pool(name="cpool", bufs=1) as cpool,
            tc.tile_pool(name="sp", bufs=3) as sp,
            tc.tile_pool(name="sp2", bufs=2) as sp2,
            tc.tile_pool(name="pm", bufs=2, space="PSUM") as pm_pool,
            tc.tile_pool(name="pe", bufs=2, space="PSUM") as pe_pool,
            tc.tile_pool(name="pwin", bufs=2, space="PSUM") as pwin_pool,
            tc.tile_pool(name="paux", bufs=2, space="PSUM") as paux_pool,
        ):
            # ---- load consts ----
            C = {}
            for k, (sh, dt) in cshapes.items():
                C[k] = cpool.tile(sh, dt, tag=f"c_{k}", name=f"c_{k}")
                nc.sync.dma_start(out=C[k][:], in_=t_c[k][:])

            lhsT_sb = cpool.tile([D, NW, D], bf16, tag="lhsT_sb", name="lhsT_sb")
            hT_res = cpool.tile([D, HT], bf16, tag="hT_res", name="hT_res")
            y_sb = cpool.tile([1, HT], f32, tag="y_sb", name="y_sb")

            def dense_table(layer, t_tab):
                wl = C[f"wl{layer}"]
                if layer == 1:
                    srcs = [(None, 0, N)]
                else:
                    srcs = [(c8, c8 * NPC, NPC) for c8 in range(ncores)]
                for c8, gbase, nrows in srcs:
                    for r0 in range(0, nrows, 128):
                        rn = min(128, nrows - r0)
                        xt_t = sp.tile([D, 128], bf16, tag="xt_t", name="xt_t")
                        if layer == 1:
                            nc.sync.dma_start(out=xt_t[:, :rn], in_=t_xT[:, r0 : r0 + rn])
                        else:
                            nc.sync.dma_start(
                                out=xt_t[:, :rn], in_=t_h1T_all[c8, :, r0 : r0 + rn]
                            )
                        ps = paux_pool.tile([D, 129], f32, tag="paux", name="ps")
                        nc.tensor.matmul(
                            out=ps[:rn, :128], lhsT=xt_t[:, :rn], rhs=wl[:, :],
                            start=True, stop=True,
                        )
                        stg = sp.tile([D, D], bf16, tag="stg", name="stg")
                        nc.scalar.copy(out=stg[:rn, :], in_=ps[:rn, :128])
                        nc.sync.dma_start(
                            out=t_tab[gbase + r0 : gbase + r0 + rn, :], in_=stg[:rn, :]
                        )

            def dense_xr(layer):
                wr = C[f"wr{layer}"]
                for w in range(NW):
                    wn = min(WIN, NPC - w * WIN)
                    if layer == 1:
                        xt_t = sp.tile([D, WIN], bf16, tag="xt_w", name="xt_w")
                        nc.sync.dma_start(
                            out=xt_t[:, :wn], in_=t_xT_own[:, w * WIN : w * WIN + wn]
                        )
                        lhs = xt_t[:, :wn]
                    else:
                        lhs = hT_res[:, w * WIN : w * WIN + wn]
                    ps = paux_pool.tile([D, 129], f32, tag="paux", name="psx")
                    nc.tensor.matmul(
                        out=ps[:wn, :128], lhsT=lhs, rhs=wr[:, :],
                        start=True, stop=True,
                    )
                    if wn < WIN:
                        # partition ranges must start aligned; clear the whole
                        # window then overwrite the live rows
                        nc.vector.memset(lhsT_sb[0:WIN, w, 0:128], 0.0)
                    nc.scalar.copy(out=lhsT_sb[:wn, w, 0:128], in_=ps[:wn, :128])
                    nc.sync.dma_start(
                        out=lhsT_sb[124:128, w, 0:128], in_=t_c[f"we{layer}"][:, :]
                    )

            # HW Lrelu is table-interpolated and costs ~5x the rel-err; the
            # two-pass Copy+Relu score path is the default.
            use_lrelu = os.environ.get("GNN_LRELU", "") != ""

            def edge_pass(layer, t_tab):
                attabs = C[f"attabs{layer}"]
                sgnc = C[f"sgnc{layer}"]
                att02 = C[f"att02_{layer}"]
                sgn08 = C[f"sgn08_{layer}"]
                for w in range(NW):
                    kwlo, kwhi = KWLO[w], KWHI[w]
                    kw = kwlo + kwhi
                    ew = kw * 128
                    wn = min(WIN, NPC - w * WIN)
                    ko = koff[w]
                    it = sp2.tile([D, 8 * KWMAX], i16, tag="it", name="it")
                    nc.sync.dma_start(
                        out=it[:, : 8 * kw],
                        in_=t_blobI[:, 8 * ko : 8 * ko + 8 * kw],
                    )
                    rhsR = sp2.tile([D, EWMAX], bf16, tag="rhsR", name="rhsR")
                    nc.sync.dma_start(
                        out=rhsR[:, :ew], in_=t_blobR[:, 128 * ko : 128 * ko + ew]
                    )
                    oneh = sp2.tile([D, 124 * KWMAX], bf16, tag="oneh", name="oneh")
                    nc.sync.dma_start(
                        out=oneh[:, : 124 * kw],
                        in_=t_blobO[:, 124 * ko : 124 * (ko + kw)],
                    )
                    xg = sp2.tile([D, KWMAX, D], bf16, tag="xg", name="xg")
                    CH = 8  # blocks per dma_gather call (1024 idxs max safe)

                    def do_gathers(base_blk, nblk, tab_ap, icol0):
                        for g0 in range(0, nblk, CH):
                            gn = min(CH, nblk - g0)
                            nc.gpsimd.dma_gather(
                                out_ap=xg[:, base_blk + g0 : base_blk + g0 + gn, :],
                                in_ap=tab_ap,
                                idxs_ap=it[:, icol0 + 8 * g0 : icol0 + 8 * (g0 + gn)],
                                num_idxs=gn * 128,
                                num_idxs_reg=gn * 128,
                                elem_size=D,
                            )

                    do_gathers(0, kwlo, t_tab[0:half, :], 0)
                    if kwhi:
                        do_gathers(kwlo, kwhi, t_tab[half:N, :], 8 * kwlo)
                    pwin = pwin_pool.tile([D, 129], f32, tag="pwin", name="pwin")
                    nblk_done = 0
                    for t0 in range(0, kw, 4):
                        nb = min(4, kw - t0)
                        T = nb * 128
                        c0 = t0 * 128
                        # m = xr[dst] + ea@We (+ xl[src] via transposes)
                        pm = pm_pool.tile([D, 512], f32, tag="pm", name="pm")
                        nc.tensor.matmul(
                            out=pm[:, :T], lhsT=lhsT_sb[:, w, :],
                            rhs=rhsR[:, c0 : c0 + T], start=True, stop=False,
                        )
                        for cb in range(nb):
                            # regular matmul with identity rhs == transpose,
                            # but accumulates into fp32 PSUM (is_transpose
                            # requires out dtype == lhsT dtype)
                            nc.tensor.matmul(
                                out=pm[:, cb * 128 : (cb + 1) * 128],
                                lhsT=xg[:, t0 + cb, :],
                                rhs=C["identb"][:, :],
                                start=False, stop=(cb == nb - 1),
                            )
                        pev = pe_pool.tile([D, 4], f32, tag="pe", name="pev")
                        if use_lrelu:
                            # lr = Lrelu(|att| * m)  (leaky with slope NEG)
                            lr = sp.tile([D, 512], bf16, tag="lr", name="lr")
                            nc.scalar.activation(
                                out=lr[:, :T], in_=pm[:, :T], func=Act.Lrelu,
                                scale=attabs[:, :], alpha=NEG,
                            )
                            # e_t = sgn(att)^T lr_blk ; ee = exp(e)
                            for cb in range(nb):
                                nc.tensor.matmul(
                                    out=pev[:, cb : cb + 1],
                                    lhsT=lr[:, cb * 128 : (cb + 1) * 128],
                                    rhs=sgnc[:, :],
                                    start=True, stop=True,
                                )
                        else:
                            # za = 0.2*att*m ; zr = relu(|att|*m)
                            za = sp.tile([D, 512], bf16, tag="lr", name="za")
                            nc.scalar.activation(
                                out=za[:, :T], in_=pm[:, :T], func=Act.Copy,
                                scale=att02[:, :],
                            )
                            zr = sp.tile([D, 512], bf16, tag="zr", name="zr")
                            nc.scalar.activation(
                                out=zr[:, :T], in_=pm[:, :T], func=Act.Relu,
                                scale=attabs[:, :],
                            )
                            for cb in range(nb):
                                nc.tensor.matmul(
                                    out=pev[:, cb : cb + 1],
                                    lhsT=za[:, cb * 128 : (cb + 1) * 128],
                                    rhs=C["onecb"][:, :],
                                    start=True, stop=False,
                                )
                                nc.tensor.matmul(
                                    out=pev[:, cb : cb + 1],
                                    lhsT=zr[:, cb * 128 : (cb + 1) * 128],
                                    rhs=sgn08[:, :],
                                    start=False, stop=True,
                                )
                        ee = sp.tile([D, 4], f32, tag="ee", name="ee")
                        nc.scalar.activation(
                            out=ee[:, :nb], in_=pev[:, :nb], func=Act.Exp,
                        )
                        # xgs = [ee * xl[src] | ee]
                        xgs = sp.tile([D, 4, 129], bf16, tag="xgs", name="xgs")
                        for cb in range(nb):
                            nc.vector.tensor_scalar(
                                out=xgs[:, cb, 0:128], in0=xg[:, t0 + cb, :],
                                scalar1=ee[:, cb : cb + 1], scalar2=None,
                                op0=Alu.mult,
                            )
                        nc.vector.tensor_scalar(
                            out=xgs[:, 0:nb, 128], in0=ee[:, :nb],
                            scalar1=1.0, scalar2=None, op0=Alu.mult,
                        )
                        # aggregation (+ denominator in col 128)
                        for cb in range(nb):
                            glob_b = nblk_done + cb
                            nc.tensor.matmul(
                                out=pwin[0:WIN, 0:129],
                                lhsT=oneh[:, (t0 + cb) * 124 : (t0 + cb + 1) * 124],
                                rhs=xgs[:, cb, :],
                                start=(glob_b == 0), stop=(glob_b == kw - 1),
                            )
                        nblk_done += nb
                    # ---- window epilogue ----
                    rec = sp.tile([WIN, 1], f32, tag="rec", name="rec")
                    nc.vector.reciprocal(out=rec[:, :], in_=pwin[0:WIN, 128:129])
                    hw_ = sp.tile([WIN, D], f32, tag="hw", name="hw_")
                    nc.scalar.activation(
                        out=hw_[:, :], in_=pwin[0:WIN, 0:128], func=Act.Copy,
                        scale=rec[:, :],
                    )
                    nc.vector.tensor_tensor(
                        out=hw_[:, :], in0=hw_[:, :], in1=C[f"bb{layer}"][:, :],
                        op=Alu.add,
                    )
                    # ELU - 1: relu(h) + exp(min(h,0)); the -1 is folded into
                    # the next consumer (layer1: explicit -1; layer2: bfc).
                    tmin = sp.tile([WIN, D], f32, tag="tmin", name="tmin")
                    nc.vector.tensor_scalar(
                        out=tmin[:, :], in0=hw_[:, :], scalar1=0.0, scalar2=None,
                        op0=Alu.min,
                    )
                    uexp = sp.tile([WIN, D], f32, tag="uexp", name="uexp")
                    nc.scalar.activation(
                        out=uexp[:, :], in_=tmin[:, :], func=Act.Exp,
                    )
                    nc.vector.tensor_tensor(
                        out=hw_[:, :], in0=hw_[:, :], in1=tmin[:, :], op=Alu.subtract
                    )
                    if layer == 1:
                        # h1 = relu + exp(min) - 1
                        nc.vector.tensor_scalar(
                            out=uexp[:, :], in0=uexp[:, :], scalar1=-1.0,
                            scalar2=None, op0=Alu.add,
                        )
                    nc.vector.tensor_tensor(
                        out=hw_[:, :], in0=hw_[:, :], in1=uexp[:, :], op=Alu.add
                    )
                    # transpose h window -> [128f, 124]
                    pt = paux_pool.tile([D, 129], f32, tag="paux", name="pt")
                    nc.tensor.matmul(
                        out=pt[:, 0:WIN], lhsT=hw_[:, :], rhs=C["identf"][0:WIN, 0:WIN],
                        is_transpose=True, start=True, stop=True,
                    )
                    if layer == 1:
                        nc.scalar.copy(
                            out=hT_res[:, w * WIN : w * WIN + WIN], in_=pt[:, 0:WIN]
                        )
                    else:
                        h2t = sp.tile([D, WIN], bf16, tag="h2t", name="h2t")
                        nc.scalar.copy(out=h2t[:, :], in_=pt[:, 0:WIN])
                        xt_f = sp.tile([D, WIN], bf16, tag="xt_fin", name="xt_f")
                        nc.sync.dma_start(
                            out=xt_f[:, :wn], in_=t_xT_own[:, w * WIN : w * WIN + wn]
                        )
                        nc.vector.tensor_tensor(
                            out=h2t[:, :wn], in0=h2t[:, :wn], in1=xt_f[:, :wn],
                            op=Alu.add,
                        )
                        py = paux_pool.tile([D, 129], f32, tag="paux", name="py")
                        nc.tensor.matmul(
                            out=py[0:1, :wn], lhsT=C["wfc"][:, :], rhs=h2t[:, :wn],
                            start=True, stop=True,
                        )
                        nc.scalar.activation(
                            out=y_sb[:, w * WIN : w * WIN + wn], in_=py[0:1, :wn],
                            func=Act.Copy, bias=float(bfc_adj),
                        )

            # ---------------- phases (GNN_MAXPHASE truncates for bisect) ----
            maxphase = int(os.environ.get("GNN_MAXPHASE", "6"))

            def body():
                if maxphase < 6:
                    nc.vector.memset(y_sb[:, :], 0.0)
                dense_table(1, t_tab1)
                if maxphase >= 1:
                    dense_xr(1)
                if maxphase >= 2:
                    edge_pass(1, t_tab1)
                    nc.sync.dma_start(out=t_h1T_own[:, :], in_=hT_res[:, 0:NPC])
                if maxphase >= 3:
                    if ncores > 1:
                        nc.gpsimd.collective_compute(
                            "AllGather",
                            mybir.AluOpType.bypass,
                            replica_groups=[list(range(ncores))],
                            ins=[t_h1T_own[:, :]],
                            outs=[t_h1T_all[:, :, :]],
                        )
                    else:
                        nc.sync.dma_start(out=t_h1T_all[0, :, :], in_=t_h1T_own[:, :])
                if maxphase >= 4:
                    dense_table(2, t_tab2)
                if maxphase >= 5:
                    dense_xr(2)
                if maxphase >= 6:
                    edge_pass(2, t_tab2)

            body()
            nc.sync.dma_start(out=t_y[:, 0], in_=y_sb[0:1, 0:NPC])

    nc.compile()
    return nc


# ----------------------------------------------------------------------------
# entry points
# ----------------------------------------------------------------------------
def prepare(inputs, ncores=8):
    x = np.asarray(inputs["x"], np.float32)
    sched, blobI, blobR, blobO = build_host_data(
        x, inputs["edge_index"], inputs["edge_attr"], ncores
    )
    consts = build_consts(inputs)
    # fold ELU's -1 for layer 2 into the fc bias: y = (h2 - 1 + x)@Wfc + bfc
    wfc_sum = float(np.asarray(inputs["Wfc"], np.float64).sum())
    bfc_adj = float(np.asarray(inputs["bfc"]).reshape(-1)[0]) - wfc_sum
    nc = build_program(sched, bfc_adj)
    NPC = sched["NPC"]
    in_maps = []
    for c in range(ncores):
        m = dict(consts)
        m["xT_own"] = np.ascontiguousarray(consts["xT"][:, c * NPC : (c + 1) * NPC])
        m["blobI"] = np.ascontiguousarray(blobI[c])
        m["blobR"] = np.ascontiguousarray(blobR[c])
        m["blobO"] = np.ascontiguousarray(blobO[c])
        in_maps.append(m)
    return nc, in_maps, sched


def kernel(**inputs) -> np.ndarray:
    ncores = 8
    nc, in_maps, sched = prepare(inputs, ncores)
    from concourse.bass_utils import run_bass_kernel_spmd

    res = run_bass_kernel_spmd(nc, in_maps, core_ids=list(range(ncores)))
    y = np.concatenate([res.results[c]["y"] for c in range(ncores)], axis=0)
    return y.astype(np.float32)


# revision 14
# speedup vs baseline: 2.2547x; 1.0643x over previous
# Trainium2 Bass kernel for DirectionalStockGNN (2-layer GATv2 + residual head).
#
# Sharding: edges are sorted by destination node on the host; each of the 8
# cores owns a contiguous range of N/8 destination nodes and all edges into
# them.  The segment softmax is then fully core-local (scores stay bounded,
# ~|e|<15, so no max-subtraction is needed).  Node features / weights are
# replicated; the only collective is an AllGather of the layer-1 hidden state
# between the two GAT layers.
#
# All matmul operands are bf16 (PSUM accumulation fp32).  Per-core edge
# pipeline (feature-major, window = 124 consecutive dst nodes, block = 128
# edges, group = 4 blocks):
#   pm[f,t]   = xr[dst_t,f] + (ea@We)[t,f]    one matmul per group, stationary
#                                              lhsT=[xr_win;We], rhs = blobR
#                                              (host-built [onehot_dt ; ea^T])
#             + xl[src_t,f]                    identity-rhs matmuls of
#                                              dma_gather rows (transpose)
#   za/zr     = 0.2*att*m / relu(|att|*m)      two ACT passes per group
#   e_t       = ones^T za + (0.8 sgn)^T zr     two matmuls per block
#   ee        = Exp(e)                         ACT per group
#   xgs       = ee_t * [xl[src_t] | 1]         one DVE op per block [128,129]
#                                              (gather table has a ones col)
#   pwin[d,:]+= onehot_td_blk^T @ xgs          one matmul per block; col 128
#                                              accumulates the softmax denom
# Window results are staged to SBUF; the reciprocal/bias/ELU epilogue runs
# once per layer over all windows (avoids per-window engine ping-pong).

import math
import os

import numpy as np
import ml_dtypes

BF16 = ml_dtypes.bfloat16

D = 128
DE = 4
TW = 256  # gather-table row width (col 0:128 = x@Wl, col 128 = 1.0)
WIN = 124
NEG = 0.2
HALF = 25000  # gather-table split row (int16 index range)


# ----------------------------------------------------------------------------
# host-side schedule + blob construction
# ----------------------------------------------------------------------------
def _wrap16(idx):
    """dma_gather index layout: [128, n/16] int16, wrap-16, replicated x8."""
    n = idx.shape[0]
    assert n % 16 == 0
    iw = np.zeros((16, n // 16), np.int16)
    iw[np.arange(n) % 16, np.arange(n) // 16] = idx
    return np.tile(iw, (8, 1))  # [128, n//16]


def build_host_data(x, edge_index, edge_attr, ncores):
    N = x.shape[0]
    src0 = np.asarray(edge_index[0], dtype=np.int64)
    dst0 = np.asarray(edge_index[1], dtype=np.int64)
    ea = np.asarray(edge_attr, dtype=np.float32)

    # self loops with mean edge_attr per dst (PyG fill_value='mean')
    sums = np.zeros((N, DE), np.float32)
    np.add.at(sums, dst0, ea)
    cnts = np.bincount(dst0, minlength=N).astype(np.float32)
    loop_attr = sums / np.maximum(cnts, 1.0)[:, None]

    src = np.concatenate([src0, np.arange(N, dtype=np.int64)])
    dst = np.concatenate([dst0, np.arange(N, dtype=np.int64)])
    eaa = np.concatenate([ea, loop_attr], axis=0)

    order = np.argsort(dst, kind="stable")
    src_s = src[order]
    dst_s = dst[order]
    ea_s = eaa[order]

    NPC = N // ncores
    NW = math.ceil(NPC / WIN)
    half = min(HALF, N)

    # per-core window edge ranges (common window grid)
    starts = np.minimum(np.arange(NW + 1) * WIN, NPC)
    bounds = np.empty((ncores, NW + 1), np.int64)
    for c in range(ncores):
        bounds[c] = np.searchsorted(dst_s, c * NPC + starts)

    # per (core, window): split edges into src<HALF and src>=HALF
    nlo = np.empty((ncores, NW), np.int64)
    nhi = np.empty((ncores, NW), np.int64)
    for c in range(ncores):
        for w in range(NW):
            lo, hi = bounds[c, w], bounds[c, w + 1]
            nlo[c, w] = int((src_s[lo:hi] < half).sum())
            nhi[c, w] = int(hi - lo - nlo[c, w])
    KWLO = np.ceil(nlo.max(axis=0) / 128.0).astype(np.int64)
    KWHI = np.ceil(nhi.max(axis=0) / 128.0).astype(np.int64)
    KWLO = np.maximum(KWLO, 1)  # >=1 so every window has at least one block

    KW = (KWLO + KWHI).astype(np.int64)
    koff = np.zeros(NW + 1, np.int64)  # cumulative blocks
    for w in range(NW):
        koff[w + 1] = koff[w] + int(KW[w])
    KTOT = int(koff[NW])

    blobI = np.zeros((ncores, 128, 8 * KTOT), np.int16)
    blobR = np.zeros((ncores, 128, 128 * KTOT), BF16)  # [onehot_dt ; ea^T]
    blobO = np.zeros((ncores, 128, 124 * KTOT), BF16)  # onehot_td
    drng = np.arange(WIN)
    for c in range(ncores):
        for w in range(NW):
            lo, hi = bounds[c, w], bounds[c, w + 1]
            kwlo, kwhi = int(KWLO[w]), int(KWHI[w])
            kw = kwlo + kwhi
            ew = kw * 128
            ko = int(koff[w])
            base = c * NPC + w * WIN
            sw = src_s[lo:hi]
            dw = (dst_s[lo:hi] - base).astype(np.int64)
            ew_ = ea_s[lo:hi]
            mlo = sw < half
            # low half then high half, each padded to its block count
            srcp = np.zeros(ew, np.int64)
            drel = np.full(ew, 127, np.int64)  # pad marker (no onehot row)
            eap = np.zeros((ew, DE), np.float32)
            a = int(mlo.sum())
            srcp[:a] = sw[mlo]
            drel[:a] = dw[mlo]
            eap[:a] = ew_[mlo]
            b0 = kwlo * 128
            b = int((~mlo).sum())
            srcp[b0 : b0 + b] = sw[~mlo]
            drel[b0 : b0 + b] = dw[~mlo]
            eap[b0 : b0 + b] = ew_[~mlo]
            srcp[b0 + b :] = half  # high-half pads -> rel idx 0
            ilo = _wrap16(srcp[:b0].astype(np.int16))  # [128, 8*kwlo]
            if kwhi:
                ihi = _wrap16((srcp[b0:] - half).astype(np.int16))
                blobI[c, :, 8 * ko : 8 * (ko + kw)] = np.concatenate(
                    [ilo, ihi], axis=1
                )
            else:
                blobI[c, :, 8 * ko : 8 * (ko + kw)] = ilo
            # onehot (both orientations) + ea rows
            oh = (drel[None, :] == drng[:, None]).astype(np.float32)  # [124,ew]
            rblk = np.zeros((128, ew), np.float32)
            rblk[0:WIN, :] = oh
            rblk[WIN : WIN + DE, :] = eap.T
            blobR[c, :, 128 * ko : 128 * ko + ew] = rblk.astype(BF16)
            # [t, d] orientation, per block contiguous: [128, kw*124]
            ot = np.ascontiguousarray(
                oh.T.reshape(kw, 128, WIN).transpose(1, 0, 2).reshape(128, kw * WIN)
            )
            blobO[c, :, 124 * ko : 124 * (ko + kw)] = ot.astype(BF16)

    sched = dict(
        N=N, NPC=NPC, NW=NW,
        KWLO=[int(k) for k in KWLO], KWHI=[int(k) for k in KWHI],
        koff=[int(v) for v in koff], ncores=ncores, half=half,
    )
    return sched, blobI, blobR, blobO


def build_consts(ins):
    f32 = np.float32
    x = np.ascontiguousarray(np.asarray(ins["x"], f32))
    consts = {}
    consts["xT"] = np.ascontiguousarray(x.T.astype(BF16))  # [128, N] bf16
    for li in (1, 2):
        Wl = np.asarray(ins[f"W{li}l"], f32)
        Wr = np.asarray(ins[f"W{li}r"], f32)
        We = np.asarray(ins[f"W{li}e"], f32)
        a = np.asarray(ins[f"att{li}"], f32)
        consts[f"wl{li}"] = np.ascontiguousarray(Wl.astype(BF16))
        consts[f"wr{li}"] = np.ascontiguousarray(Wr.astype(BF16))
        consts[f"we{li}"] = np.ascontiguousarray(We.astype(BF16))  # [4,128]
        consts[f"attabs{li}"] = np.ascontiguousarray(np.abs(a)[:, None])  # f32
        consts[f"att02_{li}"] = np.ascontiguousarray(NEG * a[:, None])  # f32
        consts[f"sgn08_{li}"] = np.ascontiguousarray(
            ((1.0 - NEG) * np.sign(a))[:, None].astype(BF16)
        )
        b = np.asarray(ins[f"b{li}"], f32)
        consts[f"bb{li}"] = np.ascontiguousarray(np.tile(b[None, :], (WIN, 1)))
    consts["wfc"] = np.ascontiguousarray(
        np.asarray(ins["Wfc"], f32).reshape(D, 1).astype(BF16)
    )
    consts["onecb"] = np.ones((D, 1), BF16)
    consts["identb"] = np.eye(D, dtype=BF16)
    consts["identf"] = np.eye(D, dtype=np.float32)
    return consts


# ----------------------------------------------------------------------------
# bass program
# ----------------------------------------------------------------------------
def build_program(sched, bfc_adj):
    import concourse.bacc as bacc
    import concourse.bass as bass
    import concourse.mybir as mybir
    import concourse.tile as tile

    f32 = mybir.dt.float32
    bf16 = mybir.dt.bfloat16
    i16 = mybir.dt.int16
    Alu = mybir.AluOpType
    Act = mybir.ActivationFunctionType

    ncores = sched["ncores"]
    N, NPC, NW = sched["N"], sched["NPC"], sched["NW"]
    KWLO, KWHI = sched["KWLO"], sched["KWHI"]
    koff = sched["koff"]
    half = sched["half"]
    KW = [KWLO[w] + KWHI[w] for w in range(NW)]
    KWMAX = max(KW)
    EWMAX = KWMAX * 128
    HT = NW * WIN
    KTOT = koff[NW]

    nc = bacc.Bacc(
        "TRN2", target_bir_lowering=False, debug=False,
        enable_asserts=False, num_devices=ncores,
    )

    # ---- I/O ----
    t_xT = nc.dram_tensor("xT", [D, N], bf16, kind="ExternalInput")
    t_xT_own = nc.dram_tensor("xT_own", [D, NPC], bf16, kind="ExternalInput")
    t_blobI = nc.dram_tensor("blobI", [128, 8 * KTOT], i16, kind="ExternalInput")
    t_blobR = nc.dram_tensor("blobR", [128, 128 * KTOT], bf16, kind="ExternalInput")
    t_blobO = nc.dram_tensor("blobO", [128, 124 * KTOT], bf16, kind="ExternalInput")
    cshapes = dict(
        wl1=([D, D], bf16), wr1=([D, D], bf16),
        wl2=([D, D], bf16), wr2=([D, D], bf16),
        we1=([DE, D], bf16), we2=([DE, D], bf16),
        attabs1=([D, 1], f32), att02_1=([D, 1], f32), sgn08_1=([D, 1], bf16),
        attabs2=([D, 1], f32), att02_2=([D, 1], f32), sgn08_2=([D, 1], bf16),
        bb1=([WIN, D], f32), bb2=([WIN, D], f32),
        wfc=([D, 1], bf16), onecb=([D, 1], bf16),
        identb=([D, D], bf16), identf=([D, D], f32),
    )
    t_c = {k: nc.dram_tensor(k, sh, dt, kind="ExternalInput")
           for k, (sh, dt) in cshapes.items()}
    t_y = nc.dram_tensor("y", [NPC, 1], f32, kind="ExternalOutput")

    # ---- DRAM internals ----
    t_tab1 = nc.dram_tensor("tab1", [N, TW], bf16, kind="Internal")
    t_tab2 = nc.dram_tensor("tab2", [N, TW], bf16, kind="Internal")
    t_h1T_own = nc.dram_tensor("h1T_own", [D, NPC], bf16, kind="Internal")
    t_h1T_all = nc.dram_tensor(
        "h1T_all", [ncores, D, NPC], bf16, kind="Internal",
        addr_space=("Shared" if ncores > 1 else "Local"),
    )

    with tile.TileContext(nc) as tc:
        with (
            tc.tile_pool(name="cpool", bufs=1) as cpool,
            tc.tile_pool(name="sp", bufs=3) as sp,
            tc.tile_pool(name="sp2", bufs=3) as sp2,
            tc.tile_pool(name="spg", bufs=2) as spg,
            tc.tile_pool(name="pm", bufs=2, space="PSUM") as pm_pool,
            tc.tile_pool(name="pe", bufs=2, space="PSUM") as pe_pool,
            tc.tile_pool(name="pwin", bufs=2, space="PSUM") as pwin_pool,
            tc.tile_pool(name="paux", bufs=2, space="PSUM") as paux_pool,
        ):
            # ---- load consts ----
            C = {}
            for k, (sh, dt) in cshapes.items():
                C[k] = cpool.tile(sh, dt, tag=f"c_{k}", name=f"c_{k}")
                nc.sync.dma_start(out=C[k][:], in_=t_c[k][:])

            lhsT_sb = cpool.tile([D, NW, D], bf16, tag="lhsT_sb", name="lhsT_sb")
            hT_res = cpool.tile([D, HT], bf16, tag="hT_res", name="hT_res")
            # all gather indices, loaded once (same for both layers)
            itall = cpool.tile([128, 8 * KTOT], i16, tag="itall", name="itall")
            nc.sync.dma_start(out=itall[:, :], in_=t_blobI[:, :])
            # per-window aggregation results staged for the batched epilogue
            stage = cpool.tile([WIN, NW, 129], f32, tag="stage", name="stage")
            scratch = cpool.tile([WIN, NW, D], f32, tag="scratch", name="scr")

            def dense_table(layer, t_tab):
                wl = C[f"wl{layer}"]
                if layer == 1:
                    srcs = [(None, 0, N)]
                else:
                    srcs = [(c8, c8 * NPC, NPC) for c8 in range(ncores)]
                for c8, gbase, nrows in srcs:
                    for r0 in range(0, nrows, 512):
                        rn = min(512, nrows - r0)
                        nch = math.ceil(rn / 128)
                        xt4 = sp.tile([D, 512], bf16, tag="xt4", name="xt4")
                        if layer == 1:
                            nc.sync.dma_start(
                                out=xt4[:, :rn], in_=t_xT[:, r0 : r0 + rn]
                            )
                        else:
                            nc.sync.dma_start(
                                out=xt4[:, :rn], in_=t_h1T_all[c8, :, r0 : r0 + rn]
                            )
                        for c in range(nch):
                            cn = min(128, rn - c * 128)
                            ps = paux_pool.tile([D, 129], f32, tag="paux", name="ps")
                            nc.tensor.matmul(
                                out=ps[:cn, :128],
                                lhsT=xt4[:, c * 128 : c * 128 + cn],
                                rhs=wl[:, :], start=True, stop=True,
                            )
                            stg = sp.tile([D, 129], bf16, tag="stg", name="stg")
                            nc.scalar.copy(out=stg[:cn, 0:128], in_=ps[:cn, :128])
                            nc.vector.memset(stg[:cn, 128:129], 1.0)
                            g0 = gbase + r0 + c * 128
                            nc.sync.dma_start(
                                out=t_tab[g0 : g0 + cn, 0:129], in_=stg[:cn, :]
                            )

            def dense_xr(layer):
                wr = C[f"wr{layer}"]
                for w in range(NW):
                    wn = min(WIN, NPC - w * WIN)
                    if layer == 1:
                        xt_t = sp.tile([D, WIN], bf16, tag="xt_w", name="xt_w")
                        nc.sync.dma_start(
                            out=xt_t[:, :wn], in_=t_xT_own[:, w * WIN : w * WIN + wn]
                        )
                        lhs = xt_t[:, :wn]
                    else:
                        lhs = hT_res[:, w * WIN : w * WIN + wn]
                    ps = paux_pool.tile([D, 129], f32, tag="paux", name="psx")
                    nc.tensor.matmul(
                        out=ps[:wn, :128], lhsT=lhs, rhs=wr[:, :],
                        start=True, stop=True,
                    )
                    if wn < WIN:
                        # partition ranges must start aligned; clear the whole
                        # window then overwrite the live rows
                        nc.vector.memset(lhsT_sb[0:WIN, w, 0:128], 0.0)
                    nc.scalar.copy(out=lhsT_sb[:wn, w, 0:128], in_=ps[:wn, :128])
                    nc.sync.dma_start(
                        out=lhsT_sb[124:128, w, 0:128], in_=t_c[f"we{layer}"][:, :]
                    )

            def edge_pass(layer, t_tab):
                attabs = C[f"attabs{layer}"]
                att02 = C[f"att02_{layer}"]
                sgn08 = C[f"sgn08_{layer}"]
                for w in range(NW):
                    kwlo, kwhi = KWLO[w], KWHI[w]
                    kw = kwlo + kwhi
                    ew = kw * 128
                    ko = koff[w]
                    rhsR = sp2.tile([D, EWMAX], bf16, tag="rhsR", name="rhsR")
                    nc.sync.dma_start(
                        out=rhsR[:, :ew], in_=t_blobR[:, 128 * ko : 128 * ko + ew]
                    )
                    oneh = sp2.tile([D, 124 * KWMAX], bf16, tag="oneh", name="oneh")
                    nc.sync.dma_start(
                        out=oneh[:, : 124 * kw],
                        in_=t_blobO[:, 124 * ko : 124 * (ko + kw)],
                    )
                    xg = spg.tile([D, KWMAX, TW], bf16, tag="xg", name="xg")
                    CH = 8  # blocks per dma_gather call (1024 idxs max safe)

                    def do_gathers(base_blk, nblk, tab_ap, icol0):
                        for g0 in range(0, nblk, CH):
                            gn = min(CH, nblk - g0)
                            nc.gpsimd.dma_gather(
                                out_ap=xg[:, base_blk + g0 : base_blk + g0 + gn, :],
                                in_ap=tab_ap,
                                idxs_ap=itall[
                                    :, 8 * (ko + icol0 + g0) : 8 * (ko + icol0 + g0 + gn)
                                ],
                                num_idxs=gn * 128,
                                num_idxs_reg=gn * 128,
                                elem_size=TW,
                            )

                    do_gathers(0, kwlo, t_tab[0:half, :], 0)
                    if kwhi:
                        do_gathers(kwlo, kwhi, t_tab[half:N, :], kwlo)
                    pwin = pwin_pool.tile([D, 129], f32, tag="pwin", name="pwin")
                    nblk_done = 0
                    for t0 in range(0, kw, 4):
                        nb = min(4, kw - t0)
                        T = nb * 128
                        c0 = t0 * 128
                        # m = xr[dst] + ea@We (+ xl[src] via identity matmuls)
                        pm = pm_pool.tile([D, 512], f32, tag="pm", name="pm")
                        nc.tensor.matmul(
                            out=pm[:, :T], lhsT=lhsT_sb[:, w, :],
                            rhs=rhsR[:, c0 : c0 + T], start=True, stop=False,
                        )
                        for cb in range(nb):
                            # regular matmul with identity rhs == transpose,
                            # but accumulates into fp32 PSUM
                            nc.tensor.matmul(
                                out=pm[:, cb * 128 : (cb + 1) * 128],
                                lhsT=xg[:, t0 + cb, 0:128],
                                rhs=C["identb"][:, :],
                                start=False, stop=(cb == nb - 1),
                            )
                        # za = 0.2*att*m ; zr = relu(|att|*m)
                        za = sp.tile([D, 512], bf16, tag="za", name="za")
                        nc.scalar.activation(
                            out=za[:, :T], in_=pm[:, :T], func=Act.Copy,
                            scale=att02[:, :],
                        )
                        zr = sp.tile([D, 512], bf16, tag="zr", name="zr")
                        nc.scalar.activation(
                            out=zr[:, :T], in_=pm[:, :T], func=Act.Relu,
                            scale=attabs[:, :],
                        )
                        pev = pe_pool.tile([D, 4], f32, tag="pe", name="pev")
                        for cb in range(nb):
                            nc.tensor.matmul(
                                out=pev[:, cb : cb + 1],
                                lhsT=za[:, cb * 128 : (cb + 1) * 128],
                                rhs=C["onecb"][:, :],
                                start=True, stop=False,
                            )
                            nc.tensor.matmul(
                                out=pev[:, cb : cb + 1],
                                lhsT=zr[:, cb * 128 : (cb + 1) * 128],
                                rhs=sgn08[:, :],
                                start=False, stop=True,
                            )
                        ee = sp.tile([D, 4], f32, tag="ee", name="ee")
                        nc.scalar.activation(
                            out=ee[:, :nb], in_=pev[:, :nb], func=Act.Exp,
                        )
                        # xgs = ee * [xl[src] | 1]  (table col 128 is 1.0)
                        xgs = sp.tile([D, 4, 129], bf16, tag="xgs", name="xgs")
                        for cb in range(nb):
                            nc.vector.tensor_scalar(
                                out=xgs[:, cb, :], in0=xg[:, t0 + cb, 0:129],
                                scalar1=ee[:, cb : cb + 1], scalar2=None,
                                op0=Alu.mult,
                            )
                        # aggregation (+ denominator in col 128)
                        for cb in range(nb):
                            glob_b = nblk_done + cb
                            nc.tensor.matmul(
                                out=pwin[0:WIN, 0:129],
                                lhsT=oneh[:, (t0 + cb) * 124 : (t0 + cb + 1) * 124],
                                rhs=xgs[:, cb, :],
                                start=(glob_b == 0), stop=(glob_b == kw - 1),
                            )
                        nblk_done += nb
                    # stage the window result; epilogue runs once per layer
                    nc.scalar.copy(out=stage[:, w, :], in_=pwin[0:WIN, 0:129])

            def epilogue(layer):
                bb = C[f"bb{layer}"]
                # rec = 1/den for all windows
                recs = sp.tile([WIN, NW], f32, tag="recs", name="recs")
                nc.vector.reciprocal(out=recs[:, :], in_=stage[:, :, 128])
                # h = num * rec + b   (per-window scalar mult, then one big add)
                for w in range(NW):
                    nc.vector.tensor_scalar(
                        out=stage[:, w, 0:128], in0=stage[:, w, 0:128],
                        scalar1=recs[:, w : w + 1], scalar2=None, op0=Alu.mult,
                    )
                for w in range(NW):
                    nc.vector.tensor_tensor(
                        out=stage[:, w, 0:128], in0=stage[:, w, 0:128],
                        in1=bb[:, :], op=Alu.add,
                    )
                # ELU - 1 = relu(h) + exp(min(h,0)) - 1; the -1 is folded into
                # the next consumer (layer1: explicit; layer2: bfc).
                nc.vector.tensor_scalar(
                    out=scratch[:, :, :], in0=stage[:, :, 0:128],
                    scalar1=0.0, scalar2=None, op0=Alu.min,
                )
                for w in range(NW):  # stage -= tmin  (= relu(h))
                    nc.vector.tensor_tensor(
                        out=stage[:, w, 0:128], in0=stage[:, w, 0:128],
                        in1=scratch[:, w, :], op=Alu.subtract,
                    )
                nc.scalar.activation(  # scratch = exp(tmin)
                    out=scratch[:, :, :], in_=scratch[:, :, :], func=Act.Exp,
                )
                if layer == 1:
                    nc.vector.tensor_scalar(
                        out=scratch[:, :, :], in0=scratch[:, :, :],
                        scalar1=-1.0, scalar2=None, op0=Alu.add,
                    )
                for w in range(NW):  # stage += exp(tmin) [- 1]
                    nc.vector.tensor_tensor(
                        out=stage[:, w, 0:128], in0=stage[:, w, 0:128],
                        in1=scratch[:, w, :], op=Alu.add,
                    )
                # transpose each window into feature-major hT_res
                for w in range(NW):
                    pt = paux_pool.tile([D, 129], f32, tag="paux", name="pt")
                    nc.tensor.matmul(
                        out=pt[:, 0:WIN], lhsT=stage[:, w, 0:128],
                        rhs=C["identf"][0:WIN, 0:WIN],
                        is_transpose=True, start=True, stop=True,
                    )
                    nc.scalar.copy(
                        out=hT_res[:, w * WIN : w * WIN + WIN], in_=pt[:, 0:WIN]
                    )

            def head():
                # y = (h2 + x) @ wfc + bfc' ; hT_res holds h2 (elu - 1 folded
                # into bfc_adj)
                for c0 in range(0, NPC, 512):
                    cn = min(512, NPC - c0)
                    xt_f = sp.tile([D, 512], bf16, tag="xt_fin", name="xt_f")
                    nc.sync.dma_start(
                        out=xt_f[:, :cn], in_=t_xT_own[:, c0 : c0 + cn]
                    )
                    h2c = sp.tile([D, 512], bf16, tag="h2c", name="h2c")
                    nc.vector.tensor_tensor(
                        out=h2c[:, :cn], in0=hT_res[:, c0 : c0 + cn],
                        in1=xt_f[:, :cn], op=Alu.add,
                    )
                    for q0 in range(0, cn, 128):
                        qn = min(128, cn - q0)
                        py = paux_pool.tile([D, 129], f32, tag="paux", name="py")
                        nc.tensor.matmul(
                            out=py[0:1, :qn], lhsT=C["wfc"][:, :],
                            rhs=h2c[:, q0 : q0 + qn], start=True, stop=True,
                        )
                        ych = sp.tile([1, 128], f32, tag="ych", name="ych")
                        nc.scalar.activation(
                            out=ych[:, :qn], in_=py[0:1, :qn],
                            func=Act.Copy, bias=float(bfc_adj),
                        )
                        nc.sync.dma_start(
                            out=t_y[c0 + q0 : c0 + q0 + qn, 0], in_=ych[0:1, :qn]
                        )

            # ---------------- phases (GNN_MAXPHASE truncates for bisect) ----
            maxphase = int(os.environ.get("GNN_MAXPHASE", "7"))

            dense_table(1, t_tab1)
            if maxphase >= 1:
                dense_xr(1)
            if maxphase >= 2:
                edge_pass(1, t_tab1)
                epilogue(1)
                nc.sync.dma_start(out=t_h1T_own[:, :], in_=hT_res[:, 0:NPC])
            if maxphase >= 3:
                if ncores > 1:
                    nc.gpsimd.collective_compute(
                        "AllGather",
                        mybir.AluOpType.bypass,
                        replica_groups=[list(range(ncores))],
                        ins=[t_h1T_own[:, :]],
                        outs=[t_h1T_all[:, :, :]],
                    )
                else:
                    nc.sync.dma_start(out=t_h1T_all[0, :, :], in_=t_h1T_own[:, :])
            if maxphase >= 4:
                dense_table(2, t_tab2)
            if maxphase >= 5:
                dense_xr(2)
            if maxphase >= 6:
                edge_pass(2, t_tab2)
                epilogue(2)
            if maxphase >= 7:
                head()

    nc.compile()
    return nc


# ----------------------------------------------------------------------------
# entry points
# ----------------------------------------------------------------------------
def prepare(inputs, ncores=8):
    x = np.asarray(inputs["x"], np.float32)
    sched, blobI, blobR, blobO = build_host_data(
        x, inputs["edge_index"], inputs["edge_attr"], ncores
    )
    consts = build_consts(inputs)
    # fold ELU's -1 for layer 2 into the fc bias: y = (h2 - 1 + x)@Wfc + bfc
    wfc_sum = float(np.asarray(inputs["Wfc"], np.float64).sum())
    bfc_adj = float(np.asarray(inputs["bfc"]).reshape(-1)[0]) - wfc_sum
    nc = build_program(sched, bfc_adj)
    NPC = sched["NPC"]
    in_maps = []
    for c in range(ncores):
        m = dict(consts)
        m["xT_own"] = np.ascontiguousarray(consts["xT"][:, c * NPC : (c + 1) * NPC])
        m["blobI"] = np.ascontiguousarray(blobI[c])
        m["blobR"] = np.ascontiguousarray(blobR[c])
        m["blobO"] = np.ascontiguousarray(blobO[c])
        in_maps.append(m)
    return nc, in_maps, sched


def kernel(**inputs) -> np.ndarray:
    ncores = 8
    nc, in_maps, sched = prepare(inputs, ncores)
    from concourse.bass_utils import run_bass_kernel_spmd

    res = run_bass_kernel_spmd(nc, in_maps, core_ids=list(range(ncores)))
    y = np.concatenate([res.results[c]["y"] for c in range(ncores)], axis=0)
    return y.astype(np.float32)


# revision 26
# speedup vs baseline: 2.5811x; 1.1447x over previous
# Trainium2 Bass kernel for DirectionalStockGNN (2-layer GATv2 + residual head).
#
# Sharding: edges are sorted by destination node on the host; each of the 8
# cores owns a contiguous range of N/8 destination nodes and all edges into
# them.  The segment softmax is then fully core-local (scores stay bounded,
# ~|e|<15, so no max-subtraction is needed).  Node features / weights are
# replicated; the only collective is an AllGather of the layer-1 hidden state
# between the two GAT layers.
#
# All matmul operands are bf16 (PSUM accumulation fp32).  Per-core edge
# pipeline (feature-major, window = 124 consecutive dst nodes, block = 128
# edges, group = 4 blocks):
#   pm[f,t]   = xr[dst_t,f] + (ea@We)[t,f]    one matmul per group, stationary
#                                              lhsT=[xr_win;We], rhs = blobR
#                                              (host-built [onehot_dt ; ea^T])
#             + xl[src_t,f]                    identity-rhs matmuls of
#                                              dma_gather rows (transpose)
#   za/zr     = 0.2*att*m / relu(|att|*m)      two ACT passes per group
#   e_t       = ones^T za + (0.8 sgn)^T zr     two matmuls per block
#   ee        = Exp(e)                         ACT per group
#   xgs       = ee_t * [xl[src_t] | 1]         one DVE op per block [128,129]
#                                              (gather table has a ones col)
#   pwin[d,:]+= onehot_td_blk^T @ xgs          one matmul per block; col 128
#                                              accumulates the softmax denom
# Window results are staged to SBUF; the reciprocal/bias/ELU epilogue runs
# once per layer over all windows (avoids per-window engine ping-pong).

import math
import os

import numpy as np
import ml_dtypes

BF16 = ml_dtypes.bfloat16

D = 128
DE = 4
TW = 256  # gather-table row width (col 0:128 = x@Wl, col 128 = 1.0)
WIN = 124
NEG = 0.2
HALF = 25000  # gather-table split row (int16 index range)


# ----------------------------------------------------------------------------
# host-side schedule + blob construction
# ----------------------------------------------------------------------------
def _wrap16(idx):
    """dma_gather index layout: [128, n/16] int16, wrap-16, replicated x8."""
    n = idx.shape[0]
    assert n % 16 == 0
    iw = np.zeros((16, n // 16), np.int16)
    iw[np.arange(n) % 16, np.arange(n) // 16] = idx
    return np.tile(iw, (8, 1))  # [128, n//16]


def build_host_data(x, edge_index, edge_attr, ncores):
    N = x.shape[0]
    src0 = np.asarray(edge_index[0], dtype=np.int64)
    dst0 = np.asarray(edge_index[1], dtype=np.int64)
    ea = np.asarray(edge_attr, dtype=np.float32)

    # self loops with mean edge_attr per dst (PyG fill_value='mean')
    sums = np.zeros((N, DE), np.float32)
    np.add.at(sums, dst0, ea)
    cnts = np.bincount(dst0, minlength=N).astype(np.float32)
    loop_attr = sums / np.maximum(cnts, 1.0)[:, None]

    src = np.concatenate([src0, np.arange(N, dtype=np.int64)])
    dst = np.concatenate([dst0, np.arange(N, dtype=np.int64)])
    eaa = np.concatenate([ea, loop_attr], axis=0)

    order = np.argsort(dst, kind="stable")
    src_s = src[order]
    dst_s = dst[order]
    ea_s = eaa[order]

    NPC = N // ncores
    NW = math.ceil(NPC / WIN)
    half = min(HALF, N)
    NCHLO = math.ceil(half / 128)
    NCHHI = max(1, math.ceil((N - half) / 128))

    # per-core window edge ranges (common window grid)
    starts = np.minimum(np.arange(NW + 1) * WIN, NPC)
    bounds = np.empty((ncores, NW + 1), np.int64)
    for c in range(ncores):
        bounds[c] = np.searchsorted(dst_s, c * NPC + starts)

    # per (core, window): split edges into src<HALF and src>=HALF
    nlo = np.empty((ncores, NW), np.int64)
    nhi = np.empty((ncores, NW), np.int64)
    for c in range(ncores):
        for w in range(NW):
            lo, hi = bounds[c, w], bounds[c, w + 1]
            nlo[c, w] = int((src_s[lo:hi] < half).sum())
            nhi[c, w] = int(hi - lo - nlo[c, w])
    KWLO = np.ceil(nlo.max(axis=0) / 128.0).astype(np.int64)
    KWHI = np.ceil(nhi.max(axis=0) / 128.0).astype(np.int64)
    KWLO = np.maximum(KWLO, 1)  # >=1 so every window has at least one block

    KW = (KWLO + KWHI).astype(np.int64)
    koff = np.zeros(NW + 1, np.int64)  # cumulative blocks
    for w in range(NW):
        koff[w + 1] = koff[w] + int(KW[w])
    KTOT = int(koff[NW])

    blobI = np.zeros((ncores, 128, 8 * KTOT), np.int16)
    blobR = np.zeros((ncores, 128, 128 * KTOT), BF16)  # [onehot_dt ; ea^T]
    blobO = np.zeros((ncores, 128, 124 * KTOT), BF16)  # onehot_td
    drng = np.arange(WIN)
    for c in range(ncores):
        for w in range(NW):
            lo, hi = bounds[c, w], bounds[c, w + 1]
            kwlo, kwhi = int(KWLO[w]), int(KWHI[w])
            kw = kwlo + kwhi
            ew = kw * 128
            ko = int(koff[w])
            base = c * NPC + w * WIN
            sw = src_s[lo:hi]
            dw = (dst_s[lo:hi] - base).astype(np.int64)
            ew_ = ea_s[lo:hi]
            mlo = sw < half
            # low half then high half, each padded to its block count
            srcp = np.zeros(ew, np.int64)
            drel = np.full(ew, 127, np.int64)  # pad marker (no onehot row)
            eap = np.zeros((ew, DE), np.float32)
            a = int(mlo.sum())
            srcp[:a] = sw[mlo]
            drel[:a] = dw[mlo]
            eap[:a] = ew_[mlo]
            b0 = kwlo * 128
            b = int((~mlo).sum())
            srcp[b0 : b0 + b] = sw[~mlo]
            drel[b0 : b0 + b] = dw[~mlo]
            eap[b0 : b0 + b] = ew_[~mlo]
            srcp[b0 + b :] = half  # high-half pads -> rel idx 0
            # permuted indices for the partition-major [128, NCH, TW] tables:
            # node i lives at (i%128, i//128) -> flat row (i%128)*NCH + i//128
            vlo = srcp[:b0]
            ilo = _wrap16(((vlo % 128) * NCHLO + vlo // 128).astype(np.int16))
            if kwhi:
                vhi = srcp[b0:] - half
                ihi = _wrap16(((vhi % 128) * NCHHI + vhi // 128).astype(np.int16))
                blobI[c, :, 8 * ko : 8 * (ko + kw)] = np.concatenate(
                    [ilo, ihi], axis=1
                )
            else:
                blobI[c, :, 8 * ko : 8 * (ko + kw)] = ilo
            # onehot (both orientations) + ea rows
            oh = (drel[None, :] == drng[:, None]).astype(np.float32)  # [124,ew]
            rblk = np.zeros((128, ew), np.float32)
            rblk[0:WIN, :] = oh
            rblk[WIN : WIN + DE, :] = eap.T
            blobR[c, :, 128 * ko : 128 * ko + ew] = rblk.astype(BF16)
            # [t, d] orientation, per block contiguous: [128, kw*124]
            ot = np.ascontiguousarray(
                oh.T.reshape(kw, 128, WIN).transpose(1, 0, 2).reshape(128, kw * WIN)
            )
            blobO[c, :, 124 * ko : 124 * (ko + kw)] = ot.astype(BF16)

    sched = dict(
        N=N, NPC=NPC, NW=NW,
        KWLO=[int(k) for k in KWLO], KWHI=[int(k) for k in KWHI],
        koff=[int(v) for v in koff], ncores=ncores, half=half,
    )
    return sched, blobI, blobR, blobO


def build_consts(ins):
    f32 = np.float32
    x = np.ascontiguousarray(np.asarray(ins["x"], f32))
    consts = {}
    consts["xT"] = np.ascontiguousarray(x.T.astype(BF16))  # [128, N] bf16
    for li in (1, 2):
        Wl = np.asarray(ins[f"W{li}l"], f32)
        Wr = np.asarray(ins[f"W{li}r"], f32)
        We = np.asarray(ins[f"W{li}e"], f32)
        a = np.asarray(ins[f"att{li}"], f32)
        consts[f"wl{li}"] = np.ascontiguousarray(Wl.astype(BF16))
        consts[f"wr{li}"] = np.ascontiguousarray(Wr.astype(BF16))
        consts[f"we{li}"] = np.ascontiguousarray(We.astype(BF16))  # [4,128]
        consts[f"attabs{li}"] = np.ascontiguousarray(np.abs(a)[:, None])  # f32
        consts[f"att02_{li}"] = np.ascontiguousarray(NEG * a[:, None])  # f32
        consts[f"sgn08_{li}"] = np.ascontiguousarray(
            ((1.0 - NEG) * np.sign(a))[:, None].astype(BF16)
        )
        b = np.asarray(ins[f"b{li}"], f32)
        consts[f"bb{li}"] = np.ascontiguousarray(np.tile(b[None, :], (WIN, 1)))
    consts["wfc"] = np.ascontiguousarray(
        np.asarray(ins["Wfc"], f32).reshape(D, 1).astype(BF16)
    )
    consts["onecb"] = np.ones((D, 1), BF16)
    consts["identb"] = np.eye(D, dtype=BF16)
    consts["identf"] = np.eye(D, dtype=np.float32)
    return consts


# ----------------------------------------------------------------------------
# bass program
# ----------------------------------------------------------------------------
def build_program(sched, bfc_adj):
    import concourse.bacc as bacc
    import concourse.bass as bass
    import concourse.mybir as mybir
    import concourse.tile as tile

    f32 = mybir.dt.float32
    bf16 = mybir.dt.bfloat16
    i16 = mybir.dt.int16
    Alu = mybir.AluOpType
    Act = mybir.ActivationFunctionType

    ncores = sched["ncores"]
    N, NPC, NW = sched["N"], sched["NPC"], sched["NW"]
    KWLO, KWHI = sched["KWLO"], sched["KWHI"]
    koff = sched["koff"]
    half = sched["half"]
    KW = [KWLO[w] + KWHI[w] for w in range(NW)]
    KWMAX = max(KW)
    EWMAX = KWMAX * 128
    HT = NW * WIN
    KTOT = koff[NW]

    nc = bacc.Bacc(
        "TRN2", target_bir_lowering=False, debug=False,
        enable_asserts=False, num_devices=ncores,
    )

    # ---- I/O ----
    t_xT = nc.dram_tensor("xT", [D, N], bf16, kind="ExternalInput")
    t_xT_own = nc.dram_tensor("xT_own", [D, NPC], bf16, kind="ExternalInput")
    t_blobI = nc.dram_tensor("blobI", [128, 8 * KTOT], i16, kind="ExternalInput")
    t_blobR = nc.dram_tensor("blobR", [128, 128 * KTOT], bf16, kind="ExternalInput")
    t_blobO = nc.dram_tensor("blobO", [128, 124 * KTOT], bf16, kind="ExternalInput")
    cshapes = dict(
        wl1=([D, D], bf16), wr1=([D, D], bf16),
        wl2=([D, D], bf16), wr2=([D, D], bf16),
        we1=([DE, D], bf16), we2=([DE, D], bf16),
        attabs1=([D, 1], f32), att02_1=([D, 1], f32), sgn08_1=([D, 1], bf16),
        attabs2=([D, 1], f32), att02_2=([D, 1], f32), sgn08_2=([D, 1], bf16),
        bb1=([WIN, D], f32), bb2=([WIN, D], f32),
        wfc=([D, 1], bf16), onecb=([D, 1], bf16),
        identb=([D, D], bf16), identf=([D, D], f32),
    )
    t_c = {k: nc.dram_tensor(k, sh, dt, kind="ExternalInput")
           for k, (sh, dt) in cshapes.items()}
    t_y = nc.dram_tensor("y", [NPC, 1], f32, kind="ExternalOutput")

    # ---- DRAM internals ----
    # gather tables are partition-major ([128, NCH, TW], node i at
    # (i%128, i//128)) so dense_table can write 4 chunks per DMA
    NCHLO = math.ceil(half / 128)
    NCHHI = max(1, math.ceil((N - half) / 128))
    t_tab1lo = nc.dram_tensor("tab1lo", [128, NCHLO, TW], bf16, kind="Internal")
    t_tab1hi = nc.dram_tensor("tab1hi", [128, NCHHI, TW], bf16, kind="Internal")
    t_tab2lo = nc.dram_tensor("tab2lo", [128, NCHLO, TW], bf16, kind="Internal")
    t_tab2hi = nc.dram_tensor("tab2hi", [128, NCHHI, TW], bf16, kind="Internal")
    t_h1T_own = nc.dram_tensor("h1T_own", [D, NPC], bf16, kind="Internal")
    t_h1T_all = nc.dram_tensor(
        "h1T_all", [ncores, D, NPC], bf16, kind="Internal",
        addr_space=("Shared" if ncores > 1 else "Local"),
    )

    with tile.TileContext(nc) as tc:
        with (
            tc.tile_pool(name="cpool", bufs=1) as cpool,
            tc.tile_pool(name="sp", bufs=3) as sp,
            tc.tile_pool(name="sp2", bufs=3) as sp2,
            tc.tile_pool(name="spg", bufs=3) as spg,
            tc.tile_pool(name="pm", bufs=2, space="PSUM") as pm_pool,
            tc.tile_pool(name="pe", bufs=2, space="PSUM") as pe_pool,
            tc.tile_pool(name="pwin", bufs=2, space="PSUM") as pwin_pool,
            tc.tile_pool(name="paux", bufs=2, space="PSUM") as paux_pool,
        ):
            # ---- load consts ----
            C = {}
            for k, (sh, dt) in cshapes.items():
                C[k] = cpool.tile(sh, dt, tag=f"c_{k}", name=f"c_{k}")
                nc.sync.dma_start(out=C[k][:], in_=t_c[k][:])

            lhsT_sb = cpool.tile([D, NW, D], bf16, tag="lhsT_sb", name="lhsT_sb")
            hT_res = cpool.tile([D, HT], bf16, tag="hT_res", name="hT_res")
            # all gather indices, loaded once (same for both layers)
            itall = cpool.tile([128, 8 * KTOT], i16, tag="itall", name="itall")
            nc.sync.dma_start(out=itall[:, :], in_=t_blobI[:, :])
            # per-window aggregation results staged for the batched epilogue
            stage = cpool.tile([WIN, NW, 129], f32, tag="stage", name="stage")
            scratch = cpool.tile([WIN, NW, D], f32, tag="scratch", name="scr")

            def dense_table(layer, t_lo, t_hi):
                wl = C[f"wl{layer}"]
                halves = [(t_lo, 0, half)]
                if N > half:
                    halves.append((t_hi, half, N - half))
                for t_tabh, gbase, nrows in halves:
                    for r0 in range(0, nrows, 512):
                        rn = min(512, nrows - r0)
                        nch = math.ceil(rn / 128)
                        xt4 = sp.tile([D, 512], bf16, tag="xt4", name="xt4")
                        if layer == 1:
                            nc.sync.dma_start(
                                out=xt4[:, :rn],
                                in_=t_xT[:, gbase + r0 : gbase + r0 + rn],
                            )
                        else:
                            # source pieces split at h1T_all core boundaries
                            off = 0
                            while off < rn:
                                g = gbase + r0 + off
                                c8 = g // NPC
                                take = min(rn - off, (c8 + 1) * NPC - g)
                                nc.sync.dma_start(
                                    out=xt4[:, off : off + take],
                                    in_=t_h1T_all[c8, :, g - c8 * NPC : g - c8 * NPC + take],
                                )
                                off += take
                        stg4 = sp.tile([D, 4, 129], bf16, tag="stg4", name="stg4")
                        for c in range(nch):
                            cn = min(128, rn - c * 128)
                            ps = paux_pool.tile([D, 129], f32, tag="paux", name="ps")
                            nc.tensor.matmul(
                                out=ps[:cn, :128],
                                lhsT=xt4[:, c * 128 : c * 128 + cn],
                                rhs=wl[:, :], start=True, stop=True,
                            )
                            nc.scalar.copy(out=stg4[:cn, c, 0:128], in_=ps[:cn, :128])
                        nc.vector.memset(stg4[:, 0:nch, 128], 1.0)
                        # one batched write per 4 chunks, issued off-sync;
                        # garbage rows in ragged tail chunks land in unused
                        # table slots that no gather index references
                        nc.scalar.dma_start(
                            out=t_tabh[:, r0 // 128 : r0 // 128 + nch, 0:129],
                            in_=stg4[:, 0:nch, :],
                        )

            def dense_xr(layer):
                wr = C[f"wr{layer}"]
                for w in range(NW):
                    wn = min(WIN, NPC - w * WIN)
                    if layer == 1:
                        xt_t = sp.tile([D, WIN], bf16, tag="xt_w", name="xt_w")
                        nc.sync.dma_start(
                            out=xt_t[:, :wn], in_=t_xT_own[:, w * WIN : w * WIN + wn]
                        )
                        lhs = xt_t[:, :wn]
                    else:
                        lhs = hT_res[:, w * WIN : w * WIN + wn]
                    ps = paux_pool.tile([D, 129], f32, tag="paux", name="psx")
                    nc.tensor.matmul(
                        out=ps[:wn, :128], lhsT=lhs, rhs=wr[:, :],
                        start=True, stop=True,
                    )
                    if wn < WIN:
                        # partition ranges must start aligned; clear the whole
                        # window then overwrite the live rows
                        nc.vector.memset(lhsT_sb[0:WIN, w, 0:128], 0.0)
                    nc.scalar.copy(out=lhsT_sb[:wn, w, 0:128], in_=ps[:wn, :128])
                    nc.sync.dma_start(
                        out=lhsT_sb[124:128, w, 0:128], in_=t_c[f"we{layer}"][:, :]
                    )

            def edge_pass(layer, t_lo, t_hi):
                attabs = C[f"attabs{layer}"]
                att02 = C[f"att02_{layer}"]
                sgn08 = C[f"sgn08_{layer}"]
                lo_ap = t_lo[:, :, :].rearrange("p c t -> (p c) t")
                hi_ap = t_hi[:, :, :].rearrange("p c t -> (p c) t")
                for w in range(NW):
                    kwlo, kwhi = KWLO[w], KWHI[w]
                    kw = kwlo + kwhi
                    ew = kw * 128
                    ko = koff[w]
                    rhsR = sp2.tile([D, EWMAX], bf16, tag="rhsR", name="rhsR")
                    nc.sync.dma_start(
                        out=rhsR[:, :ew], in_=t_blobR[:, 128 * ko : 128 * ko + ew]
                    )
                    oneh = sp2.tile([D, 124 * KWMAX], bf16, tag="oneh", name="oneh")
                    nc.sync.dma_start(
                        out=oneh[:, : 124 * kw],
                        in_=t_blobO[:, 124 * ko : 124 * (ko + kw)],
                    )
                    xg = spg.tile([D, KWMAX, TW], bf16, tag="xg", name="xg")
                    CH = 8  # blocks per dma_gather call (1024 idxs max safe)

                    def do_gathers(base_blk, nblk, tab_ap, icol0):
                        for g0 in range(0, nblk, CH):
                            gn = min(CH, nblk - g0)
                            nc.gpsimd.dma_gather(
                                out_ap=xg[:, base_blk + g0 : base_blk + g0 + gn, :],
                                in_ap=tab_ap,
                                idxs_ap=itall[
                                    :, 8 * (ko + icol0 + g0) : 8 * (ko + icol0 + g0 + gn)
                                ],
                                num_idxs=gn * 128,
                                num_idxs_reg=gn * 128,
                                elem_size=TW,
                            )

                    do_gathers(0, kwlo, lo_ap, 0)
                    if kwhi:
                        do_gathers(kwlo, kwhi, hi_ap, kwlo)
                    pwin = pwin_pool.tile([D, 129], f32, tag="pwin", name="pwin")
                    nblk_done = 0
                    for t0 in range(0, kw, 4):
                        nb = min(4, kw - t0)
                        T = nb * 128
                        c0 = t0 * 128
                        # m = xr[dst] + ea@We (+ xl[src] via identity matmuls)
                        pm = pm_pool.tile([D, 512], f32, tag="pm", name="pm")
                        nc.tensor.matmul(
                            out=pm[:, :T], lhsT=lhsT_sb[:, w, :],
                            rhs=rhsR[:, c0 : c0 + T], start=True, stop=False,
                        )
                        for cb in range(nb):
                            # regular matmul with identity rhs == transpose,
                            # but accumulates into fp32 PSUM
                            nc.tensor.matmul(
                                out=pm[:, cb * 128 : (cb + 1) * 128],
                                lhsT=xg[:, t0 + cb, 0:128],
                                rhs=C["identb"][:, :],
                                start=False, stop=(cb == nb - 1),
                            )
                        # za = 0.2*att*m ; zr = relu(|att|*m)  (on DVE; the
                        # ACT engine carries exp + half the xgs scales)
                        za = sp.tile([D, 512], bf16, tag="za", name="za")
                        nc.vector.tensor_scalar(
                            out=za[:, :T], in0=pm[:, :T],
                            scalar1=att02[:, :], scalar2=None, op0=Alu.mult,
                        )
                        zr = sp.tile([D, 512], bf16, tag="zr", name="zr")
                        nc.vector.tensor_scalar(
                            out=zr[:, :T], in0=pm[:, :T],
                            scalar1=attabs[:, :], scalar2=0.0,
                            op0=Alu.mult, op1=Alu.max,
                        )
                        pev = pe_pool.tile([D, 4], f32, tag="pe", name="pev")
                        for cb in range(nb):
                            nc.tensor.matmul(
                                out=pev[:, cb : cb + 1],
                                lhsT=za[:, cb * 128 : (cb + 1) * 128],
                                rhs=C["onecb"][:, :],
                                start=True, stop=False,
                            )
                            nc.tensor.matmul(
                                out=pev[:, cb : cb + 1],
                                lhsT=zr[:, cb * 128 : (cb + 1) * 128],
                                rhs=sgn08[:, :],
                                start=False, stop=True,
                            )
                        ee = sp.tile([D, 4], f32, tag="ee", name="ee")
                        nc.scalar.activation(
                            out=ee[:, :nb], in_=pev[:, :nb], func=Act.Exp,
                        )
                        # xgs = ee * [xl[src] | 1]  (table col 128 is 1.0);
                        # alternate DVE/ACT to balance engine load
                        xgs = sp.tile([D, 4, 129], bf16, tag="xgs", name="xgs")
                        for cb in range(nb):
                            if cb % 2 == 0:
                                nc.vector.tensor_scalar(
                                    out=xgs[:, cb, :], in0=xg[:, t0 + cb, 0:129],
                                    scalar1=ee[:, cb : cb + 1], scalar2=None,
                                    op0=Alu.mult,
                                )
                            else:
                                nc.scalar.activation(
                                    out=xgs[:, cb, :], in_=xg[:, t0 + cb, 0:129],
                                    func=Act.Copy, scale=ee[:, cb : cb + 1],
                                )
                        # aggregation (+ denominator in col 128)
                        for cb in range(nb):
                            glob_b = nblk_done + cb
                            nc.tensor.matmul(
                                out=pwin[0:WIN, 0:129],
                                lhsT=oneh[:, (t0 + cb) * 124 : (t0 + cb + 1) * 124],
                                rhs=xgs[:, cb, :],
                                start=(glob_b == 0), stop=(glob_b == kw - 1),
                            )
                        nblk_done += nb
                    # stage the window result; epilogue runs once per layer
                    nc.scalar.copy(out=stage[:, w, :], in_=pwin[0:WIN, 0:129])

            def epilogue(layer):
                bb = C[f"bb{layer}"]
                # rec = 1/den for all windows
                recs = sp.tile([WIN, NW], f32, tag="recs", name="recs")
                nc.vector.reciprocal(out=recs[:, :], in_=stage[:, :, 128])
                # h = num * rec + b   (per-window scalar mult, then one big add)
                for w in range(NW):
                    nc.vector.tensor_scalar(
                        out=stage[:, w, 0:128], in0=stage[:, w, 0:128],
                        scalar1=recs[:, w : w + 1], scalar2=None, op0=Alu.mult,
                    )
                for w in range(NW):
                    nc.vector.tensor_tensor(
                        out=stage[:, w, 0:128], in0=stage[:, w, 0:128],
                        in1=bb[:, :], op=Alu.add,
                    )
                # ELU - 1 = relu(h) + exp(min(h,0)) - 1; the -1 is folded into
                # the next consumer (layer1: explicit; layer2: bfc).
                nc.vector.tensor_scalar(
                    out=scratch[:, :, :], in0=stage[:, :, 0:128],
                    scalar1=0.0, scalar2=None, op0=Alu.min,
                )
                for w in range(NW):  # stage -= tmin  (= relu(h))
                    nc.vector.tensor_tensor(
                        out=stage[:, w, 0:128], in0=stage[:, w, 0:128],
                        in1=scratch[:, w, :], op=Alu.subtract,
                    )
                nc.scalar.activation(  # scratch = exp(tmin)
                    out=scratch[:, :, :], in_=scratch[:, :, :], func=Act.Exp,
                )
                if layer == 1:
                    nc.vector.tensor_scalar(
                        out=scratch[:, :, :], in0=scratch[:, :, :],
                        scalar1=-1.0, scalar2=None, op0=Alu.add,
                    )
                for w in range(NW):  # stage += exp(tmin) [- 1]
                    nc.vector.tensor_tensor(
                        out=stage[:, w, 0:128], in0=stage[:, w, 0:128],
                        in1=scratch[:, w, :], op=Alu.add,
                    )
                # transpose each window into feature-major hT_res
                for w in range(NW):
                    pt = paux_pool.tile([D, 129], f32, tag="paux", name="pt")
                    nc.tensor.matmul(
                        out=pt[:, 0:WIN], lhsT=stage[:, w, 0:128],
                        rhs=C["identf"][0:WIN, 0:WIN],
                        is_transpose=True, start=True, stop=True,
                    )
                    nc.scalar.copy(
                        out=hT_res[:, w * WIN : w * WIN + WIN], in_=pt[:, 0:WIN]
                    )

            def head():
                # y = (h2 + x) @ wfc + bfc' ; hT_res holds h2 (elu - 1 folded
                # into bfc_adj)
                for c0 in range(0, NPC, 512):
                    cn = min(512, NPC - c0)
                    xt_f = sp.tile([D, 512], bf16, tag="xt_fin", name="xt_f")
                    nc.sync.dma_start(
                        out=xt_f[:, :cn], in_=t_xT_own[:, c0 : c0 + cn]
                    )
                    h2c = sp.tile([D, 512], bf16, tag="h2c", name="h2c")
                    nc.vector.tensor_tensor(
                        out=h2c[:, :cn], in0=hT_res[:, c0 : c0 + cn],
                        in1=xt_f[:, :cn], op=Alu.add,
                    )
                    for q0 in range(0, cn, 128):
                        qn = min(128, cn - q0)
                        py = paux_pool.tile([D, 129], f32, tag="paux", name="py")
                        nc.tensor.matmul(
                            out=py[0:1, :qn], lhsT=C["wfc"][:, :],
                            rhs=h2c[:, q0 : q0 + qn], start=True, stop=True,
                        )
                        ych = sp.tile([1, 128], f32, tag="ych", name="ych")
                        nc.scalar.activation(
                            out=ych[:, :qn], in_=py[0:1, :qn],
                            func=Act.Copy, bias=float(bfc_adj),
                        )
                        nc.sync.dma_start(
                            out=t_y[c0 + q0 : c0 + q0 + qn, 0], in_=ych[0:1, :qn]
                        )

            # ---------------- phases (GNN_MAXPHASE truncates for bisect) ----
            maxphase = int(os.environ.get("GNN_MAXPHASE", "7"))

            dense_table(1, t_tab1lo, t_tab1hi)
            if maxphase >= 1:
                dense_xr(1)
            if maxphase >= 2:
                edge_pass(1, t_tab1lo, t_tab1hi)
                epilogue(1)
                nc.sync.dma_start(out=t_h1T_own[:, :], in_=hT_res[:, 0:NPC])
            if maxphase >= 3:
                if ncores > 1:
                    nc.gpsimd.collective_compute(
                        "AllGather",
                        mybir.AluOpType.bypass,
                        replica_groups=[list(range(ncores))],
                        ins=[t_h1T_own[:, :]],
                        outs=[t_h1T_all[:, :, :]],
                    )
                else:
                    nc.sync.dma_start(out=t_h1T_all[0, :, :], in_=t_h1T_own[:, :])
            if maxphase >= 4:
                dense_table(2, t_tab2lo, t_tab2hi)
            if maxphase >= 5:
                dense_xr(2)
            if maxphase >= 6:
                edge_pass(2, t_tab2lo, t_tab2hi)
                epilogue(2)
            if maxphase >= 7:
                head()

    nc.compile()
    return nc


# ----------------------------------------------------------------------------
# entry points
# ----------------------------------------------------------------------------
def prepare(inputs, ncores=8):
    x = np.asarray(inputs["x"], np.float32)
    sched, blobI, blobR, blobO = build_host_data(
        x, inputs["edge_index"], inputs["edge_attr"], ncores
    )
    consts = build_consts(inputs)
    # fold ELU's -1 for layer 2 into the fc bias: y = (h2 - 1 + x)@Wfc + bfc
    wfc_sum = float(np.asarray(inputs["Wfc"], np.float64).sum())
    bfc_adj = float(np.asarray(inputs["bfc"]).reshape(-1)[0]) - wfc_sum
    nc = build_program(sched, bfc_adj)
    NPC = sched["NPC"]
    in_maps = []
    for c in range(ncores):
        m = dict(consts)
        m["xT_own"] = np.ascontiguousarray(consts["xT"][:, c * NPC : (c + 1) * NPC])
        m["blobI"] = np.ascontiguousarray(blobI[c])
        m["blobR"] = np.ascontiguousarray(blobR[c])
        m["blobO"] = np.ascontiguousarray(blobO[c])
        in_maps.append(m)
    return nc, in_maps, sched


def kernel(**inputs) -> np.ndarray:
    ncores = 8
    nc, in_maps, sched = prepare(inputs, ncores)
    from concourse.bass_utils import run_bass_kernel_spmd

    res = run_bass_kernel_spmd(nc, in_maps, core_ids=list(range(ncores)))
    y = np.concatenate([res.results[c]["y"] for c in range(ncores)], axis=0)
    return y.astype(np.float32)


# revision 29
# speedup vs baseline: 3.0168x; 1.1688x over previous
# Trainium2 Bass kernel for DirectionalStockGNN (2-layer GATv2 + residual head).
#
# Sharding: edges are sorted by destination node on the host; each of the 8
# cores owns a contiguous range of N/8 destination nodes and all edges into
# them.  The segment softmax is then fully core-local (scores stay bounded,
# ~|e|<15, so no max-subtraction is needed).  Node features / weights are
# replicated; the only collective is an AllGather of the layer-1 hidden state
# between the two GAT layers.
#
# All matmul operands are bf16 (PSUM accumulation fp32).  Per-core edge
# pipeline (feature-major, window = 124 consecutive dst nodes, block = 128
# edges, group = 4 blocks):
#   pm[f,t]   = xr[dst_t,f] + (ea@We)[t,f]    one matmul per group, stationary
#                                              lhsT=[xr_win;We], rhs = blobR
#                                              (host-built [onehot_dt ; ea^T])
#             + xl[src_t,f]                    identity-rhs matmuls of
#                                              dma_gather rows (transpose)
#   za/zr     = 0.2*att*m / relu(|att|*m)      two ACT passes per group
#   e_t       = ones^T za + (0.8 sgn)^T zr     two matmuls per block
#   ee        = Exp(e)                         ACT per group
#   xgs       = ee_t * [xl[src_t] | 1]         one DVE op per block [128,129]
#                                              (gather table has a ones col)
#   pwin[d,:]+= onehot_td_blk^T @ xgs          one matmul per block; col 128
#                                              accumulates the softmax denom
# Window results are staged to SBUF; the reciprocal/bias/ELU epilogue runs
# once per layer over all windows (avoids per-window engine ping-pong).

import math
import os

import numpy as np
import ml_dtypes

BF16 = ml_dtypes.bfloat16

D = 128
DE = 4
TW = 256  # gather-table row width (col 0:128 = x@Wl, col 128 = 1.0)
WIN = 124
NEG = 0.2
HALF = 25000  # gather-table split row (int16 index range)


# ----------------------------------------------------------------------------
# host-side schedule + blob construction
# ----------------------------------------------------------------------------
def _wrap16(idx):
    """dma_gather index layout: [128, n/16] int16, wrap-16, replicated x8."""
    n = idx.shape[0]
    assert n % 16 == 0
    iw = np.zeros((16, n // 16), np.int16)
    iw[np.arange(n) % 16, np.arange(n) // 16] = idx
    return np.tile(iw, (8, 1))  # [128, n//16]


def build_host_data(x, edge_index, edge_attr, ncores):
    N = x.shape[0]
    src0 = np.asarray(edge_index[0], dtype=np.int64)
    dst0 = np.asarray(edge_index[1], dtype=np.int64)
    ea = np.asarray(edge_attr, dtype=np.float32)

    # self loops with mean edge_attr per dst (PyG fill_value='mean')
    sums = np.zeros((N, DE), np.float32)
    np.add.at(sums, dst0, ea)
    cnts = np.bincount(dst0, minlength=N).astype(np.float32)
    loop_attr = sums / np.maximum(cnts, 1.0)[:, None]

    src = np.concatenate([src0, np.arange(N, dtype=np.int64)])
    dst = np.concatenate([dst0, np.arange(N, dtype=np.int64)])
    eaa = np.concatenate([ea, loop_attr], axis=0)

    order = np.argsort(dst, kind="stable")
    src_s = src[order]
    dst_s = dst[order]
    ea_s = eaa[order]

    NPC = N // ncores
    NW = math.ceil(NPC / WIN)
    half = min(HALF, N)
    NCHLO = math.ceil(half / 128)
    NCHHI = max(1, math.ceil((N - half) / 128))

    # per-core window edge ranges (common window grid)
    starts = np.minimum(np.arange(NW + 1) * WIN, NPC)
    bounds = np.empty((ncores, NW + 1), np.int64)
    for c in range(ncores):
        bounds[c] = np.searchsorted(dst_s, c * NPC + starts)

    # per (core, window): split edges into src<HALF and src>=HALF
    nlo = np.empty((ncores, NW), np.int64)
    nhi = np.empty((ncores, NW), np.int64)
    for c in range(ncores):
        for w in range(NW):
            lo, hi = bounds[c, w], bounds[c, w + 1]
            nlo[c, w] = int((src_s[lo:hi] < half).sum())
            nhi[c, w] = int(hi - lo - nlo[c, w])
    KWLO = np.ceil(nlo.max(axis=0) / 128.0).astype(np.int64)
    KWHI = np.ceil(nhi.max(axis=0) / 128.0).astype(np.int64)
    KWLO = np.maximum(KWLO, 1)  # >=1 so every window has at least one block

    KW = (KWLO + KWHI).astype(np.int64)
    koff = np.zeros(NW + 1, np.int64)  # cumulative blocks
    for w in range(NW):
        koff[w + 1] = koff[w] + int(KW[w])
    KTOT = int(koff[NW])

    blobI = np.zeros((ncores, 128, 8 * KTOT), np.int16)
    blobR = np.zeros((ncores, 128, 128 * KTOT), BF16)  # [onehot_dt ; ea^T]
    blobO = np.zeros((ncores, 128, 124 * KTOT), BF16)  # onehot_td
    drng = np.arange(WIN)
    for c in range(ncores):
        for w in range(NW):
            lo, hi = bounds[c, w], bounds[c, w + 1]
            kwlo, kwhi = int(KWLO[w]), int(KWHI[w])
            kw = kwlo + kwhi
            ew = kw * 128
            ko = int(koff[w])
            base = c * NPC + w * WIN
            sw = src_s[lo:hi]
            dw = (dst_s[lo:hi] - base).astype(np.int64)
            ew_ = ea_s[lo:hi]
            mlo = sw < half
            # low half then high half, each padded to its block count
            srcp = np.zeros(ew, np.int64)
            drel = np.full(ew, 127, np.int64)  # pad marker (no onehot row)
            eap = np.zeros((ew, DE), np.float32)
            a = int(mlo.sum())
            srcp[:a] = sw[mlo]
            drel[:a] = dw[mlo]
            eap[:a] = ew_[mlo]
            b0 = kwlo * 128
            b = int((~mlo).sum())
            srcp[b0 : b0 + b] = sw[~mlo]
            drel[b0 : b0 + b] = dw[~mlo]
            eap[b0 : b0 + b] = ew_[~mlo]
            srcp[b0 + b :] = half  # high-half pads -> rel idx 0
            # permuted indices for the partition-major [128, NCH, TW] tables:
            # node i lives at (i%128, i//128) -> flat row (i%128)*NCH + i//128
            vlo = srcp[:b0]
            ilo = _wrap16(((vlo % 128) * NCHLO + vlo // 128).astype(np.int16))
            if kwhi:
                vhi = srcp[b0:] - half
                ihi = _wrap16(((vhi % 128) * NCHHI + vhi // 128).astype(np.int16))
                blobI[c, :, 8 * ko : 8 * (ko + kw)] = np.concatenate(
                    [ilo, ihi], axis=1
                )
            else:
                blobI[c, :, 8 * ko : 8 * (ko + kw)] = ilo
            # onehot (both orientations) + ea rows
            oh = (drel[None, :] == drng[:, None]).astype(np.float32)  # [124,ew]
            rblk = np.zeros((128, ew), np.float32)
            rblk[0:WIN, :] = oh
            rblk[WIN : WIN + DE, :] = eap.T
            blobR[c, :, 128 * ko : 128 * ko + ew] = rblk.astype(BF16)
            # [t, d] orientation, per block contiguous: [128, kw*124]
            ot = np.ascontiguousarray(
                oh.T.reshape(kw, 128, WIN).transpose(1, 0, 2).reshape(128, kw * WIN)
            )
            blobO[c, :, 124 * ko : 124 * (ko + kw)] = ot.astype(BF16)

    sched = dict(
        N=N, NPC=NPC, NW=NW,
        KWLO=[int(k) for k in KWLO], KWHI=[int(k) for k in KWHI],
        koff=[int(v) for v in koff], ncores=ncores, half=half,
    )
    return sched, blobI, blobR, blobO


def build_consts(ins):
    f32 = np.float32
    x = np.ascontiguousarray(np.asarray(ins["x"], f32))
    consts = {}
    consts["xT"] = np.ascontiguousarray(x.T.astype(BF16))  # [128, N] bf16
    for li in (1, 2):
        Wl = np.asarray(ins[f"W{li}l"], f32)
        Wr = np.asarray(ins[f"W{li}r"], f32)
        We = np.asarray(ins[f"W{li}e"], f32)
        a = np.asarray(ins[f"att{li}"], f32)
        consts[f"wl{li}"] = np.ascontiguousarray(Wl.astype(BF16))
        consts[f"wr{li}"] = np.ascontiguousarray(Wr.astype(BF16))
        consts[f"we{li}"] = np.ascontiguousarray(We.astype(BF16))  # [4,128]
        consts[f"attabs{li}"] = np.ascontiguousarray(np.abs(a)[:, None])  # f32
        consts[f"att02_{li}"] = np.ascontiguousarray(NEG * a[:, None])  # f32
        consts[f"sgn08_{li}"] = np.ascontiguousarray(
            ((1.0 - NEG) * np.sign(a))[:, None].astype(BF16)
        )
        b = np.asarray(ins[f"b{li}"], f32)
        consts[f"bb{li}"] = np.ascontiguousarray(np.tile(b[None, :], (WIN, 1)))
    consts["wfc"] = np.ascontiguousarray(
        np.asarray(ins["Wfc"], f32).reshape(D, 1).astype(BF16)
    )
    consts["onecb"] = np.ones((D, 1), BF16)
    consts["identb"] = np.eye(D, dtype=BF16)
    consts["identf"] = np.eye(D, dtype=np.float32)
    return consts


# ----------------------------------------------------------------------------
# bass program
# ----------------------------------------------------------------------------
def build_program(sched, bfc_adj):
    import concourse.bacc as bacc
    import concourse.bass as bass
    import concourse.mybir as mybir
    import concourse.tile as tile

    f32 = mybir.dt.float32
    bf16 = mybir.dt.bfloat16
    i16 = mybir.dt.int16
    Alu = mybir.AluOpType
    Act = mybir.ActivationFunctionType

    ncores = sched["ncores"]
    N, NPC, NW = sched["N"], sched["NPC"], sched["NW"]
    KWLO, KWHI = sched["KWLO"], sched["KWHI"]
    koff = sched["koff"]
    half = sched["half"]
    KW = [KWLO[w] + KWHI[w] for w in range(NW)]
    KWMAX = max(KW)
    EWMAX = KWMAX * 128
    HT = NW * WIN
    KTOT = koff[NW]

    nc = bacc.Bacc(
        "TRN2", target_bir_lowering=False, debug=False,
        enable_asserts=False, num_devices=ncores,
        num_swdge_queues=4,
    )

    # ---- I/O ----
    t_xT = nc.dram_tensor("xT", [D, N], bf16, kind="ExternalInput")
    t_xT_own = nc.dram_tensor("xT_own", [D, NPC], bf16, kind="ExternalInput")
    t_blobI = nc.dram_tensor("blobI", [128, 8 * KTOT], i16, kind="ExternalInput")
    t_blobR = nc.dram_tensor("blobR", [128, 128 * KTOT], bf16, kind="ExternalInput")
    t_blobO = nc.dram_tensor("blobO", [128, 124 * KTOT], bf16, kind="ExternalInput")
    cshapes = dict(
        wl1=([D, D], bf16), wr1=([D, D], bf16),
        wl2=([D, D], bf16), wr2=([D, D], bf16),
        we1=([DE, D], bf16), we2=([DE, D], bf16),
        attabs1=([D, 1], f32), att02_1=([D, 1], f32), sgn08_1=([D, 1], bf16),
        attabs2=([D, 1], f32), att02_2=([D, 1], f32), sgn08_2=([D, 1], bf16),
        bb1=([WIN, D], f32), bb2=([WIN, D], f32),
        wfc=([D, 1], bf16), onecb=([D, 1], bf16),
        identb=([D, D], bf16), identf=([D, D], f32),
    )
    t_c = {k: nc.dram_tensor(k, sh, dt, kind="ExternalInput")
           for k, (sh, dt) in cshapes.items()}
    t_y = nc.dram_tensor("y", [NPC, 1], f32, kind="ExternalOutput")

    # ---- DRAM internals ----
    # gather tables are partition-major ([128, NCH, TW], node i at
    # (i%128, i//128)) so dense_table can write 4 chunks per DMA
    NCHLO = math.ceil(half / 128)
    NCHHI = max(1, math.ceil((N - half) / 128))
    t_tab1lo = nc.dram_tensor("tab1lo", [128, NCHLO, TW], bf16, kind="Internal")
    t_tab1hi = nc.dram_tensor("tab1hi", [128, NCHHI, TW], bf16, kind="Internal")
    t_tab2lo = nc.dram_tensor("tab2lo", [128, NCHLO, TW], bf16, kind="Internal")
    t_tab2hi = nc.dram_tensor("tab2hi", [128, NCHHI, TW], bf16, kind="Internal")
    t_h1T_own = nc.dram_tensor("h1T_own", [D, NPC], bf16, kind="Internal")
    t_h1T_all = nc.dram_tensor(
        "h1T_all", [ncores, D, NPC], bf16, kind="Internal",
        addr_space=("Shared" if ncores > 1 else "Local"),
    )

    with tile.TileContext(nc) as tc:
        with (
            tc.tile_pool(name="cpool", bufs=1) as cpool,
            tc.tile_pool(name="sp", bufs=3) as sp,
            tc.tile_pool(name="sp2", bufs=3) as sp2,
            tc.tile_pool(name="spg", bufs=3) as spg,
            tc.tile_pool(name="pm", bufs=2, space="PSUM") as pm_pool,
            tc.tile_pool(name="pe", bufs=2, space="PSUM") as pe_pool,
            tc.tile_pool(name="pwin", bufs=2, space="PSUM") as pwin_pool,
            tc.tile_pool(name="paux", bufs=2, space="PSUM") as paux_pool,
        ):
            # ---- load consts ----
            C = {}
            for k, (sh, dt) in cshapes.items():
                C[k] = cpool.tile(sh, dt, tag=f"c_{k}", name=f"c_{k}")
                nc.sync.dma_start(out=C[k][:], in_=t_c[k][:])

            lhsT_sb = cpool.tile([D, NW, D], bf16, tag="lhsT_sb", name="lhsT_sb")
            hT_res = cpool.tile([D, HT], bf16, tag="hT_res", name="hT_res")
            # all gather indices, loaded once (same for both layers)
            itall = cpool.tile([128, 8 * KTOT], i16, tag="itall", name="itall")
            nc.sync.dma_start(out=itall[:, :], in_=t_blobI[:, :])
            # per-window aggregation results staged for the batched epilogue
            stage = cpool.tile([WIN, NW, 129], f32, tag="stage", name="stage")
            scratch = cpool.tile([WIN, NW, D], f32, tag="scratch", name="scr")

            def dense_table(layer, t_lo, t_hi):
                wl = C[f"wl{layer}"]
                halves = [(t_lo, 0, half)]
                if N > half:
                    halves.append((t_hi, half, N - half))
                for t_tabh, gbase, nrows in halves:
                    for r0 in range(0, nrows, 512):
                        rn = min(512, nrows - r0)
                        nch = math.ceil(rn / 128)
                        xt4 = sp.tile([D, 512], bf16, tag="xt4", name="xt4")
                        if layer == 1:
                            nc.sync.dma_start(
                                out=xt4[:, :rn],
                                in_=t_xT[:, gbase + r0 : gbase + r0 + rn],
                            )
                        else:
                            # source pieces split at h1T_all core boundaries
                            off = 0
                            while off < rn:
                                g = gbase + r0 + off
                                c8 = g // NPC
                                take = min(rn - off, (c8 + 1) * NPC - g)
                                nc.sync.dma_start(
                                    out=xt4[:, off : off + take],
                                    in_=t_h1T_all[c8, :, g - c8 * NPC : g - c8 * NPC + take],
                                )
                                off += take
                        stg4 = sp.tile([D, 4, 129], bf16, tag="stg4", name="stg4")
                        for c in range(nch):
                            cn = min(128, rn - c * 128)
                            ps = paux_pool.tile([D, 129], f32, tag="paux", name="ps")
                            nc.tensor.matmul(
                                out=ps[:cn, :128],
                                lhsT=xt4[:, c * 128 : c * 128 + cn],
                                rhs=wl[:, :], start=True, stop=True,
                            )
                            nc.scalar.copy(out=stg4[:cn, c, 0:128], in_=ps[:cn, :128])
                        nc.vector.memset(stg4[:, 0:nch, 128], 1.0)
                        # one batched write per 4 chunks, issued off-sync;
                        # garbage rows in ragged tail chunks land in unused
                        # table slots that no gather index references
                        nc.scalar.dma_start(
                            out=t_tabh[:, r0 // 128 : r0 // 128 + nch, 0:129],
                            in_=stg4[:, 0:nch, :],
                        )

            def dense_xr(layer):
                wr = C[f"wr{layer}"]
                for w in range(NW):
                    wn = min(WIN, NPC - w * WIN)
                    if layer == 1:
                        xt_t = sp.tile([D, WIN], bf16, tag="xt_w", name="xt_w")
                        nc.sync.dma_start(
                            out=xt_t[:, :wn], in_=t_xT_own[:, w * WIN : w * WIN + wn]
                        )
                        lhs = xt_t[:, :wn]
                    else:
                        lhs = hT_res[:, w * WIN : w * WIN + wn]
                    ps = paux_pool.tile([D, 129], f32, tag="paux", name="psx")
                    nc.tensor.matmul(
                        out=ps[:wn, :128], lhsT=lhs, rhs=wr[:, :],
                        start=True, stop=True,
                    )
                    if wn < WIN:
                        # partition ranges must start aligned; clear the whole
                        # window then overwrite the live rows
                        nc.vector.memset(lhsT_sb[0:WIN, w, 0:128], 0.0)
                    nc.scalar.copy(out=lhsT_sb[:wn, w, 0:128], in_=ps[:wn, :128])
                    nc.sync.dma_start(
                        out=lhsT_sb[124:128, w, 0:128], in_=t_c[f"we{layer}"][:, :]
                    )

            def edge_pass(layer, t_lo, t_hi):
                attabs = C[f"attabs{layer}"]
                att02 = C[f"att02_{layer}"]
                sgn08 = C[f"sgn08_{layer}"]
                lo_ap = t_lo[:, :, :].rearrange("p c t -> (p c) t")
                hi_ap = t_hi[:, :, :].rearrange("p c t -> (p c) t")
                qctr = [0]  # rotate SWDGE queues so desc-gen overlaps drain
                for w in range(NW):
                    kwlo, kwhi = KWLO[w], KWHI[w]
                    kw = kwlo + kwhi
                    ew = kw * 128
                    ko = koff[w]
                    rhsR = sp2.tile([D, EWMAX], bf16, tag="rhsR", name="rhsR")
                    nc.sync.dma_start(
                        out=rhsR[:, :ew], in_=t_blobR[:, 128 * ko : 128 * ko + ew]
                    )
                    oneh = sp2.tile([D, 124 * KWMAX], bf16, tag="oneh", name="oneh")
                    nc.sync.dma_start(
                        out=oneh[:, : 124 * kw],
                        in_=t_blobO[:, 124 * ko : 124 * (ko + kw)],
                    )
                    xg = spg.tile([D, KWMAX, TW], bf16, tag="xg", name="xg")
                    CH = 8  # blocks per dma_gather call (1024 idxs max safe)

                    def do_gathers(base_blk, nblk, tab_ap, icol0):
                        for g0 in range(0, nblk, CH):
                            gn = min(CH, nblk - g0)
                            nc.gpsimd.dma_gather(
                                out_ap=xg[:, base_blk + g0 : base_blk + g0 + gn, :],
                                in_ap=tab_ap,
                                idxs_ap=itall[
                                    :, 8 * (ko + icol0 + g0) : 8 * (ko + icol0 + g0 + gn)
                                ],
                                num_idxs=gn * 128,
                                num_idxs_reg=gn * 128,
                                elem_size=TW,
                                queue_num=qctr[0] % 4,
                            )
                            qctr[0] += 1

                    do_gathers(0, kwlo, lo_ap, 0)
                    if kwhi:
                        do_gathers(kwlo, kwhi, hi_ap, kwlo)
                    pwin = pwin_pool.tile([D, 129], f32, tag="pwin", name="pwin")
                    nblk_done = 0
                    for t0 in range(0, kw, 4):
                        nb = min(4, kw - t0)
                        T = nb * 128
                        c0 = t0 * 128
                        # m = xr[dst] + ea@We (+ xl[src] via identity matmuls)
                        pm = pm_pool.tile([D, 512], f32, tag="pm", name="pm")
                        nc.tensor.matmul(
                            out=pm[:, :T], lhsT=lhsT_sb[:, w, :],
                            rhs=rhsR[:, c0 : c0 + T], start=True, stop=False,
                        )
                        for cb in range(nb):
                            # regular matmul with identity rhs == transpose,
                            # but accumulates into fp32 PSUM
                            nc.tensor.matmul(
                                out=pm[:, cb * 128 : (cb + 1) * 128],
                                lhsT=xg[:, t0 + cb, 0:128],
                                rhs=C["identb"][:, :],
                                start=False, stop=(cb == nb - 1),
                            )
                        # za = 0.2*att*m ; zr = relu(|att|*m)  (on DVE; the
                        # ACT engine carries exp + half the xgs scales)
                        za = sp.tile([D, 512], bf16, tag="za", name="za")
                        nc.vector.tensor_scalar(
                            out=za[:, :T], in0=pm[:, :T],
                            scalar1=att02[:, :], scalar2=None, op0=Alu.mult,
                        )
                        zr = sp.tile([D, 512], bf16, tag="zr", name="zr")
                        nc.vector.tensor_scalar(
                            out=zr[:, :T], in0=pm[:, :T],
                            scalar1=attabs[:, :], scalar2=0.0,
                            op0=Alu.mult, op1=Alu.max,
                        )
                        pev = pe_pool.tile([D, 4], f32, tag="pe", name="pev")
                        for cb in range(nb):
                            nc.tensor.matmul(
                                out=pev[:, cb : cb + 1],
                                lhsT=za[:, cb * 128 : (cb + 1) * 128],
                                rhs=C["onecb"][:, :],
                                start=True, stop=False,
                            )
                            nc.tensor.matmul(
                                out=pev[:, cb : cb + 1],
                                lhsT=zr[:, cb * 128 : (cb + 1) * 128],
                                rhs=sgn08[:, :],
                                start=False, stop=True,
                            )
                        ee = sp.tile([D, 4], f32, tag="ee", name="ee")
                        nc.scalar.activation(
                            out=ee[:, :nb], in_=pev[:, :nb], func=Act.Exp,
                        )
                        # xgs = ee * [xl[src] | 1]  (table col 128 is 1.0);
                        # alternate DVE/ACT to balance engine load
                        xgs = sp.tile([D, 4, 129], bf16, tag="xgs", name="xgs")
                        for cb in range(nb):
                            if cb % 2 == 0:
                                nc.vector.tensor_scalar(
                                    out=xgs[:, cb, :], in0=xg[:, t0 + cb, 0:129],
                                    scalar1=ee[:, cb : cb + 1], scalar2=None,
                                    op0=Alu.mult,
                                )
                            else:
                                nc.scalar.activation(
                                    out=xgs[:, cb, :], in_=xg[:, t0 + cb, 0:129],
                                    func=Act.Copy, scale=ee[:, cb : cb + 1],
                                )
                        # aggregation (+ denominator in col 128)
                        for cb in range(nb):
                            glob_b = nblk_done + cb
                            nc.tensor.matmul(
                                out=pwin[0:WIN, 0:129],
                                lhsT=oneh[:, (t0 + cb) * 124 : (t0 + cb + 1) * 124],
                                rhs=xgs[:, cb, :],
                                start=(glob_b == 0), stop=(glob_b == kw - 1),
                            )
                        nblk_done += nb
                    # stage the window result; epilogue runs once per layer
                    nc.scalar.copy(out=stage[:, w, :], in_=pwin[0:WIN, 0:129])

            def epilogue(layer):
                bb = C[f"bb{layer}"]
                # rec = 1/den for all windows
                recs = sp.tile([WIN, NW], f32, tag="recs", name="recs")
                nc.vector.reciprocal(out=recs[:, :], in_=stage[:, :, 128])
                # h = num * rec + b   (per-window scalar mult, then one big add)
                for w in range(NW):
                    nc.vector.tensor_scalar(
                        out=stage[:, w, 0:128], in0=stage[:, w, 0:128],
                        scalar1=recs[:, w : w + 1], scalar2=None, op0=Alu.mult,
                    )
                for w in range(NW):
                    nc.vector.tensor_tensor(
                        out=stage[:, w, 0:128], in0=stage[:, w, 0:128],
                        in1=bb[:, :], op=Alu.add,
                    )
                # ELU - 1 = relu(h) + exp(min(h,0)) - 1; the -1 is folded into
                # the next consumer (layer1: explicit; layer2: bfc).
                nc.vector.tensor_scalar(
                    out=scratch[:, :, :], in0=stage[:, :, 0:128],
                    scalar1=0.0, scalar2=None, op0=Alu.min,
                )
                for w in range(NW):  # stage -= tmin  (= relu(h))
                    nc.vector.tensor_tensor(
                        out=stage[:, w, 0:128], in0=stage[:, w, 0:128],
                        in1=scratch[:, w, :], op=Alu.subtract,
                    )
                nc.scalar.activation(  # scratch = exp(tmin)
                    out=scratch[:, :, :], in_=scratch[:, :, :], func=Act.Exp,
                )
                if layer == 1:
                    nc.vector.tensor_scalar(
                        out=scratch[:, :, :], in0=scratch[:, :, :],
                        scalar1=-1.0, scalar2=None, op0=Alu.add,
                    )
                for w in range(NW):  # stage += exp(tmin) [- 1]
                    nc.vector.tensor_tensor(
                        out=stage[:, w, 0:128], in0=stage[:, w, 0:128],
                        in1=scratch[:, w, :], op=Alu.add,
                    )
                # transpose each window into feature-major hT_res
                for w in range(NW):
                    pt = paux_pool.tile([D, 129], f32, tag="paux", name="pt")
                    nc.tensor.matmul(
                        out=pt[:, 0:WIN], lhsT=stage[:, w, 0:128],
                        rhs=C["identf"][0:WIN, 0:WIN],
                        is_transpose=True, start=True, stop=True,
                    )
                    nc.scalar.copy(
                        out=hT_res[:, w * WIN : w * WIN + WIN], in_=pt[:, 0:WIN]
                    )

            def head():
                # y = (h2 + x) @ wfc + bfc' ; hT_res holds h2 (elu - 1 folded
                # into bfc_adj)
                for c0 in range(0, NPC, 512):
                    cn = min(512, NPC - c0)
                    xt_f = sp.tile([D, 512], bf16, tag="xt_fin", name="xt_f")
                    nc.sync.dma_start(
                        out=xt_f[:, :cn], in_=t_xT_own[:, c0 : c0 + cn]
                    )
                    h2c = sp.tile([D, 512], bf16, tag="h2c", name="h2c")
                    nc.vector.tensor_tensor(
                        out=h2c[:, :cn], in0=hT_res[:, c0 : c0 + cn],
                        in1=xt_f[:, :cn], op=Alu.add,
                    )
                    for q0 in range(0, cn, 128):
                        qn = min(128, cn - q0)
                        py = paux_pool.tile([D, 129], f32, tag="paux", name="py")
                        nc.tensor.matmul(
                            out=py[0:1, :qn], lhsT=C["wfc"][:, :],
                            rhs=h2c[:, q0 : q0 + qn], start=True, stop=True,
                        )
                        ych = sp.tile([1, 128], f32, tag="ych", name="ych")
                        nc.scalar.activation(
                            out=ych[:, :qn], in_=py[0:1, :qn],
                            func=Act.Copy, bias=float(bfc_adj),
                        )
                        nc.sync.dma_start(
                            out=t_y[c0 + q0 : c0 + q0 + qn, 0], in_=ych[0:1, :qn]
                        )

            # ---------------- phases (GNN_MAXPHASE truncates for bisect) ----
            maxphase = int(os.environ.get("GNN_MAXPHASE", "7"))

            dense_table(1, t_tab1lo, t_tab1hi)
            if maxphase >= 1:
                dense_xr(1)
            if maxphase >= 2:
                edge_pass(1, t_tab1lo, t_tab1hi)
                epilogue(1)
                nc.sync.dma_start(out=t_h1T_own[:, :], in_=hT_res[:, 0:NPC])
            if maxphase >= 3:
                if ncores > 1:
                    nc.gpsimd.collective_compute(
                        "AllGather",
                        mybir.AluOpType.bypass,
                        replica_groups=[list(range(ncores))],
                        ins=[t_h1T_own[:, :]],
                        outs=[t_h1T_all[:, :, :]],
                    )
                else:
                    nc.sync.dma_start(out=t_h1T_all[0, :, :], in_=t_h1T_own[:, :])
            if maxphase >= 4:
                dense_table(2, t_tab2lo, t_tab2hi)
            if maxphase >= 5:
                dense_xr(2)
            if maxphase >= 6:
                edge_pass(2, t_tab2lo, t_tab2hi)
                epilogue(2)
            if maxphase >= 7:
                head()

    nc.compile()
    return nc


# ----------------------------------------------------------------------------
# entry points
# ----------------------------------------------------------------------------
def prepare(inputs, ncores=8):
    x = np.asarray(inputs["x"], np.float32)
    sched, blobI, blobR, blobO = build_host_data(
        x, inputs["edge_index"], inputs["edge_attr"], ncores
    )
    consts = build_consts(inputs)
    # fold ELU's -1 for layer 2 into the fc bias: y = (h2 - 1 + x)@Wfc + bfc
    wfc_sum = float(np.asarray(inputs["Wfc"], np.float64).sum())
    bfc_adj = float(np.asarray(inputs["bfc"]).reshape(-1)[0]) - wfc_sum
    nc = build_program(sched, bfc_adj)
    NPC = sched["NPC"]
    in_maps = []
    for c in range(ncores):
        m = dict(consts)
        m["xT_own"] = np.ascontiguousarray(consts["xT"][:, c * NPC : (c + 1) * NPC])
        m["blobI"] = np.ascontiguousarray(blobI[c])
        m["blobR"] = np.ascontiguousarray(blobR[c])
        m["blobO"] = np.ascontiguousarray(blobO[c])
        in_maps.append(m)
    return nc, in_maps, sched


def kernel(**inputs) -> np.ndarray:
    ncores = 8
    nc, in_maps, sched = prepare(inputs, ncores)
    from concourse.bass_utils import run_bass_kernel_spmd

    res = run_bass_kernel_spmd(nc, in_maps, core_ids=list(range(ncores)))
    y = np.concatenate([res.results[c]["y"] for c in range(ncores)], axis=0)
    return y.astype(np.float32)
